# revision 5
# baseline (speedup 1.0000x reference)
# Trainium2 Bass kernel for nn_DiffusionModel_88948772700534 (gnn_message_passing).
#
# Strategy: data-parallel over batch B=16 across 8 NeuronCores (2 batches/core).
# All activations are kept feature-major on-chip ([feature_partition, token]) so
# every Linear becomes PE matmuls accumulated over K in PSUM, with bias +
# LeakyReLU (Prelu) fused into a single ScalarE activation reading PSUM.
# Matmuls run in float32r (full-rate fp32); weights are streamed per layer from
# HBM. The pairwise (i,j) tensors are processed in chunks of 4 i-rows (384
# pairs) so nothing pairwise is ever materialized in HBM; gf (the 640-dim
# concat) is never materialized at all - the f-matmul reads the edge state
# plus broadcast APs of the node state directly.
#
# sqrt and 1/x are computed as exp(0.5*ln(x)) / exp(-ln(x)) so that the whole
# kernel stays inside one ScalarE activation table set
# (natural_log_exp_and_others: exp/ln/square/identity/parametric_relu),
# avoiding ~2.7us table reloads. Softmax uses unnormalized exp (wn >= 0 is
# small) with the denominator folded in after the attention reduction.

import os
import sys
import types

for _p in ("/opt/trn_rl_repo",):
    if _p not in sys.path:
        sys.path.insert(0, _p)

# NTFF profile hook (lets BASS_TRACE=1 capture HW timing under axon).
try:
    import antenv

    if "antenv.axon_hooks" not in sys.modules:
        _hooks = types.ModuleType("antenv.axon_hooks")
        _hook_slot = [None]
        _hooks.set_axon_ntff_profile_hook = lambda h: _hook_slot.__setitem__(0, h)
        _hooks.get_axon_ntff_profile_hook = lambda: _hook_slot[0]
        sys.modules["antenv.axon_hooks"] = _hooks
        antenv.axon_hooks = _hooks
        try:
            from trn_agent_boot.trn_boot import _ntff_profile_via_ctypes

            _h = _ntff_profile_via_ctypes("/opt/axon/libaxon_pjrt.so")
            if _h is not None:
                _hooks.set_axon_ntff_profile_hook(_h)
        except Exception:
            pass
except Exception:
    pass

import numpy as np

import concourse.bass as bass
import concourse.tile as tile
from concourse import bacc, bass_utils, mybir

F32 = mybir.dt.float32
F32R = mybir.dt.float32r
I32 = mybir.dt.int32
A = mybir.ActivationFunctionType
OP = mybir.AluOpType
AX = mybir.AxisListType

B, N = 16, 96
NODE_D, EDGE_D = 20, 5
DN, DE, DC = 256, 128, 256
H, L, TMAX = 8, 4, 1000
EPS = 1e-5
ALPHA = 0.01
NCORES = 8
BLOC = B // NCORES          # batches per core
NN = N * N                  # 9216 pairs
CH = 4                      # i-rows per chunk
CF = CH * N                 # chunk free size (384)
NCHUNK = N // CH            # 24
ME_F = 512                  # free size for 128-feature edge passes
NME = NN // ME_F            # 18

LAST_RESULTS = None         # filled by kernel() for inspection (exec_time_ns etc.)


class _Packer:
    """Packs 2D slabs [h<=128, w] into one [128, cols] blob, column-major."""

    def __init__(self):
        self.cols = 0
        self.off = {}
        self.parts = []

    def add(self, name, arr):
        arr = np.asarray(arr, np.float32)
        assert arr.ndim == 2 and arr.shape[0] <= 128
        self.off[name] = (self.cols, arr.shape[1], arr.shape[0])
        self.parts.append((self.cols, arr))
        self.cols += arr.shape[1]

    def blob(self, cols=None):
        out = np.zeros((128, cols or self.cols), np.float32)
        for c, arr in self.parts:
            out[: arr.shape[0], c : c + arr.shape[1]] = arr
        return out


def _add_lin(pk, name, w):
    """Linear weight w [O, I] -> k-slabs of w.T, named name_k{k}."""
    wT = np.asarray(w, np.float32).T  # [I, O]
    I_, O = wT.shape
    nk = (I_ + 127) // 128
    for k in range(nk):
        pk.add(f"{name}_k{k}", wT[k * 128 : (k + 1) * 128, :])


def _add_bias(pk, name, v):
    """Bias/norm vector [O] -> one column per 128-block, named name (width nm)."""
    v = np.asarray(v, np.float32).reshape(-1)
    O = v.shape[0]
    nm = (O + 127) // 128
    cols = np.zeros((128, nm), np.float32)
    for m in range(nm):
        seg = v[m * 128 : (m + 1) * 128]
        cols[: seg.shape[0], m] = seg
    pk.add(name, cols)


def _pack_host(params):
    """Build the three weight blobs + offset maps from the params pytree."""
    p = {k: params[k] for k in params}

    misc = _Packer()
    _add_lin(misc, "in_n1", p["in_n"][0])
    _add_lin(misc, "in_n2", p["in_n"][2])
    w_ie1 = np.zeros((128, 8), np.float32)
    w_ie1[:, :EDGE_D] = np.asarray(p["in_e"][0], np.float32)
    _add_lin(misc, "in_e1", w_ie1)
    _add_lin(misc, "in_e2", p["in_e"][2])
    _add_lin(misc, "in_c1", p["in_c"][0])
    _add_lin(misc, "in_c2", p["in_c"][2])
    _add_lin(misc, "out_t1", p["out_t"][0])
    w_ot2 = np.zeros((8, DN), np.float32)
    w_ot2[:6] = np.asarray(p["out_t"][2], np.float32)
    _add_lin(misc, "out_t2", w_ot2)
    _add_lin(misc, "out_p1", p["out_p"][0])
    w_op2 = np.zeros((16, DN), np.float32)
    w_op2[:14] = np.asarray(p["out_p"][2], np.float32)
    _add_lin(misc, "out_p2", w_op2)
    _add_lin(misc, "out_e1", p["out_e"][0])
    w_oe2 = np.zeros((8, DE), np.float32)
    w_oe2[:EDGE_D] = np.asarray(p["out_e"][2], np.float32)
    _add_lin(misc, "out_e2", w_oe2)
    # E8: [256, 8] block-diag ones (sum w^2 within each head) -> 2 k-slabs
    e8 = np.zeros((256, 8), np.float32)
    for hd in range(256):
        e8[hd, hd // 32] = 1.0
    _add_lin(misc, "E8", e8.T)          # treat as w [8, 256]
    # Eexp: lhsT [8, 256] (broadcast head value to its 32 dims)
    _add_lin(misc, "Eexp", e8)          # w [256, 8] -> wT [8, 256] single slab
    misc.add("ones1", np.full((4, 128), 0.25, np.float32))

    laya = layb = None
    a_blobs, b_blobs = [], []
    for li in range(L):
        lp = p["layers"][li]
        mha = lp["mha"]
        pa = _Packer()
        _add_lin(pa, "f", mha["f"][0])
        _add_lin(pa, "e", mha["e"][0])
        _add_lin(pa, "v1", mha["v"][0])
        _add_lin(pa, "v2", mha["v"][2])
        _add_lin(pa, "w1", mha["w"][0])
        _add_lin(pa, "w2", mha["w"][2])
        _add_lin(pa, "m", mha["m"][0])
        _add_lin(pa, "a", mha["a"][0])
        pb2 = _Packer()
        _add_lin(pb2, "o", mha["o"][0])
        _add_lin(pb2, "mn1", lp["mn"][0])
        _add_lin(pb2, "mn2", lp["mn"][2])
        _add_lin(pb2, "me1", lp["me"][0])
        _add_lin(pb2, "me2", lp["me"][2])
        if laya is None:
            laya, layb = pa, pb2
        a_blobs.append(pa.blob())
        b_blobs.append(pb2.blob())
    layer_wa = np.stack(a_blobs)
    layer_wb = np.stack(b_blobs)

    bc = _Packer()
    # time-embedding constants: s/(2pi) and phase (0 or 0.25), packed per m-col
    scales = np.exp(np.arange(0, DC, 2, dtype=np.float64) * (-np.log(10000.0) / DC))
    sp = np.repeat(scales, 2) / (2.0 * np.pi)   # [256]
    ph = np.tile([0.0, 0.25], DC // 2)          # [256]
    _add_bias(bc, "eps", np.full(128, EPS))
    _add_bias(bc, "s2pi", sp)
    _add_bias(bc, "phase", ph)
    _add_bias(bc, "in_n_b1", p["in_n"][1])
    _add_bias(bc, "in_n_b2", p["in_n"][3])
    _add_bias(bc, "in_e_b1", p["in_e"][1])
    _add_bias(bc, "in_e_b2", p["in_e"][3])
    _add_bias(bc, "in_c_b1", p["in_c"][1])
    _add_bias(bc, "in_c_b2", p["in_c"][3])
    _add_bias(bc, "out_t_b1", p["out_t"][1])
    _add_bias(bc, "out_t_b2", p["out_t"][3])
    _add_bias(bc, "out_p_b1", p["out_p"][1])
    _add_bias(bc, "out_p_b2", p["out_p"][3])
    _add_bias(bc, "out_e_b1", p["out_e"][1])
    _add_bias(bc, "out_e_b2", p["out_e"][3])
    for li in range(L):
        lp = p["layers"][li]
        mha = lp["mha"]
        _add_bias(bc, f"l{li}_bf", mha["f"][1])
        _add_bias(bc, f"l{li}_be", mha["e"][1])
        _add_bias(bc, f"l{li}_bv1", mha["v"][1])
        _add_bias(bc, f"l{li}_bv2", mha["v"][3])
        _add_bias(bc, f"l{li}_bw1", mha["w"][1])
        _add_bias(bc, f"l{li}_bw2", mha["w"][3])
        _add_bias(bc, f"l{li}_bo", mha["o"][1])
        _add_bias(bc, f"l{li}_bm", mha["m"][1])
        _add_bias(bc, f"l{li}_ba", mha["a"][1])
        _add_bias(bc, f"l{li}_bmn1", lp["mn"][1])
        _add_bias(bc, f"l{li}_bmn2", lp["mn"][3])
        _add_bias(bc, f"l{li}_bme1", lp["me"][1])
        _add_bias(bc, f"l{li}_bme2", lp["me"][3])
        for nn_, wdt in (("n1", DN), ("n2", DE), ("n3", DN), ("n4", DE)):
            g, b_ = lp[nn_]
            _add_bias(bc, f"l{li}_{nn_}g", g)
            _add_bias(bc, f"l{li}_{nn_}b", b_)

    return misc, layer_wa, laya, layer_wb, layb, bc


def _build_program(misc_off, misc_cols, la_off, la_cols, lb_off, lb_cols, bc_off, bc_cols, taps=False):
    nc = bacc.Bacc("TRN2", debug=False)

    d_misc = nc.dram_tensor("misc_w", [128, misc_cols], F32R, kind="ExternalInput").ap()
    d_laya = nc.dram_tensor("layer_wa", [L, 128, la_cols], F32R, kind="ExternalInput").ap()
    d_layb = nc.dram_tensor("layer_wb", [L, 128, lb_cols], F32R, kind="ExternalInput").ap()
    d_bc = nc.dram_tensor("bias_c", [128, bc_cols], F32, kind="ExternalInput").ap()
    d_nodes = nc.dram_tensor("nodes_fm", [BLOC, NODE_D, N], F32R, kind="ExternalInput").ap()
    d_edges = nc.dram_tensor("edges_fm", [BLOC, 8, NN], F32R, kind="ExternalInput").ap()
    d_t = nc.dram_tensor("t_f32", [4, BLOC], F32R, kind="ExternalInput").ap()
    d_on = nc.dram_tensor("out_nodes_fm", [BLOC, NODE_D, N], F32, kind="ExternalOutput").ap()
    d_oe = nc.dram_tensor("out_edges_fm", [BLOC, EDGE_D, NN], F32, kind="ExternalOutput").ap()

    tapd = {}

    def tap(name, shape):
        if taps:
            tapd[name] = nc.dram_tensor(name, shape, F32, kind="ExternalOutput").ap()
        return tapd.get(name)

    with tile.TileContext(nc) as tc:
        with (
            tc.tile_pool(name="pw_misc", bufs=1) as pw_misc,
            tc.tile_pool(name="pw_laya", bufs=2) as pw_laya,
            tc.tile_pool(name="pw_layb", bufs=1) as pw_layb,
            tc.tile_pool(name="pb", bufs=1) as pbp,
            tc.tile_pool(name="pstate", bufs=1) as pstate,
            tc.tile_pool(name="pchunk", bufs=2) as pch,
            tc.tile_pool(name="pchunk1", bufs=1) as pch1,
            tc.tile_pool(name="psmall", bufs=2) as psm,
            tc.tile_pool(name="pseq", bufs=1) as psq,
            tc.tile_pool(name="pio", bufs=2) as pio,
            tc.tile_pool(name="pp", bufs=8, space="PSUM") as pp,
        ):
            mw = pw_misc.tile([128, misc_cols], F32R, tag="misc")
            nc.sync.dma_start(mw[:], d_misc[:])
            bcw = pbp.tile([128, bc_cols], F32, tag="bias")
            nc.sync.dma_start(bcw[:], d_bc[:])

            def MW(name, m=0, mw_width=128):
                off, w, h = misc_off[name]
                return mw[:h, off + m * mw_width : off + min((m + 1) * mw_width, w)]

            def BC(name, m=0, h=128):
                off, w, _ = bc_off[name]
                return bcw[:h, off + m : off + m + 1]

            def mm_psum(lhs_list, rhs_list, out_shape=(128, CF)):
                """Accumulating matmul chain into a fresh psum tile."""
                ps = pp.tile(list(out_shape), F32, tag="mm")
                nk = len(lhs_list)
                for k in range(nk):
                    nc.tensor.matmul(
                        ps[:], lhs_list[k], rhs_list[k],
                        start=(k == 0), stop=(k == nk - 1),
                    )
                return ps

            # ============ per-core program: loop over local batches ============
            for b in range(BLOC):
                # ---- persistent per-batch state tiles (reused across batches) --
                edges_t = pstate.tile([128, NN], F32R, tag="edges")
                ae_t = pstate.tile([128, NN], F32, tag="ae")
                nodes_t = psq.tile([128, 2, N], F32R, tag="nodes")

                # ---------------- time embedding + cond -----------------------
                t_sb = psm.tile([4, BLOC], F32R, tag="tsb")
                nc.sync.dma_start(t_sb[:], d_t[:])
                ps_t = pp.tile([128, BLOC], F32, tag="mm")
                nc.tensor.matmul(ps_t[:], MW("ones1"), t_sb[:], start=True, stop=True)
                emb = psq.tile([128, 2, BLOC], F32R, tag="emb")
                for m in range(2):
                    y = psm.tile([128, BLOC], F32, tag="y")
                    nc.scalar.activation(y[:], ps_t[:], A.Identity,
                                         bias=BC("phase", m), scale=BC("s2pi", m))
                    ki = psm.tile([128, BLOC], I32, tag="ki")
                    nc.vector.tensor_copy(ki[:], y[:])
                    kf = psm.tile([128, BLOC], F32, tag="kf")
                    nc.vector.tensor_copy(kf[:], ki[:])
                    fr = psm.tile([128, BLOC], F32, tag="fr")
                    nc.vector.tensor_sub(fr[:], y[:], kf[:])
                    nc.scalar.activation(emb[:, m, :], fr[:], A.Sin, scale=float(2 * np.pi))
                cmid = psq.tile([128, 2, BLOC], F32R, tag="cmid")
                for m in range(2):
                    ps = mm_psum([MW("in_c1_k0", m), MW("in_c1_k1", m)],
                                 [emb[:, 0, :], emb[:, 1, :]], (128, BLOC))
                    nc.scalar.activation(cmid[:, m, :], ps[:], A.Prelu,
                                         bias=BC("in_c_b1", m), alpha=ALPHA)
                cond = psq.tile([128, 2, BLOC], F32R, tag="cond")
                for m in range(2):
                    ps = mm_psum([MW("in_c2_k0", m), MW("in_c2_k1", m)],
                                 [cmid[:, 0, :], cmid[:, 1, :]], (128, BLOC))
                    nc.scalar.activation(cond[:, m, :], ps[:], A.Prelu,
                                         bias=BC("in_c_b2", m), alpha=ALPHA)
                if b == 0:
                    t_ = tap("tap_cond", [128, 2, BLOC])
                    if t_ is not None:
                        nc.sync.dma_start(t_[:], cond[:])

                # ---------------- input MLPs ---------------------------------
                nraw = psq.tile([NODE_D, N], F32R, tag="nraw")
                nc.sync.dma_start(nraw[:], d_nodes[b])
                nmid = psq.tile([128, 2, N], F32R, tag="nmid")
                for m in range(2):
                    ps = mm_psum([MW("in_n1_k0", m)], [nraw[:]], (128, N))
                    nc.scalar.activation(nmid[:, m, :], ps[:], A.Prelu,
                                         bias=BC("in_n_b1", m), alpha=ALPHA)
                for m in range(2):
                    ps = mm_psum([MW("in_n2_k0", m), MW("in_n2_k1", m)],
                                 [nmid[:, 0, :], nmid[:, 1, :]], (128, N))
                    nc.scalar.activation(nodes_t[:, m, :], ps[:], A.Prelu,
                                         bias=BC("in_n_b2", m), alpha=ALPHA)

                for s in range(NME):
                    sl = slice(s * ME_F, (s + 1) * ME_F)
                    eraw = pio.tile([8, ME_F], F32R, tag="pio_e")
                    nc.sync.dma_start(eraw[:], d_edges[b][:, sl])
                    ps = mm_psum([MW("in_e1_k0")], [eraw[:]], (128, ME_F))
                    emid = pio.tile([128, ME_F], F32R, tag="pio_r")
                    nc.scalar.activation(emid[:], ps[:], A.Prelu,
                                         bias=BC("in_e_b1"), alpha=ALPHA)
                    ps = mm_psum([MW("in_e2_k0")], [emid[:]], (128, ME_F))
                    nc.scalar.activation(edges_t[:, sl], ps[:], A.Prelu,
                                         bias=BC("in_e_b2"), alpha=ALPHA)

                if b == 0:
                    t_ = tap("tap_nodes_in", [128, 2, N])
                    if t_ is not None:
                        nc.sync.dma_start(t_[:], nodes_t[:])
                    t_ = tap("tap_edges_in", [128, 512])
                    if t_ is not None:
                        nc.sync.dma_start(t_[:], edges_t[:, 0:512])

                # ================= layers =====================================
                for li in range(L):
                    lwa = pw_laya.tile([128, la_cols], F32R, tag="laywa")
                    nc.sync.dma_start(lwa[:], d_laya[li])
                    lwb = pw_layb.tile([128, lb_cols], F32R, tag="laywb")
                    nc.sync.dma_start(lwb[:], d_layb[li])

                    def LW(name, m=0, _la=lwa, _lb=lwb):
                        if name.split("_k")[0] in ("o", "mn1", "mn2", "me1", "me2"):
                            off, w, h = lb_off[name]
                            return _lb[:h, off + m * 128 : off + min((m + 1) * 128, w)]
                        off, w, h = la_off[name]
                        return _la[:h, off + m * 128 : off + min((m + 1) * 128, w)]

                    pfx = f"l{li}_"

                    # ---- FiLM params: mul/add = cond @ Wm/Wa + b -------------
                    mul_sb = psm.tile([128, 2, BLOC], F32, tag="mul")
                    filmb = psm.tile([128, 2, BLOC], F32, tag="filmb")
                    add_sb = psm.tile([128, 2, BLOC], F32, tag="adds")
                    for m in range(2):
                        ps = mm_psum([LW("m_k0", m), LW("m_k1", m)],
                                     [cond[:, 0, :], cond[:, 1, :]], (128, BLOC))
                        nc.scalar.activation(mul_sb[:, m, :], ps[:], A.Identity,
                                             bias=BC(pfx + "bm", m))
                        ps = mm_psum([LW("a_k0", m), LW("a_k1", m)],
                                     [cond[:, 0, :], cond[:, 1, :]], (128, BLOC))
                        nc.scalar.activation(add_sb[:, m, :], ps[:], A.Identity,
                                             bias=BC(pfx + "ba", m))
                        # filmb = bf * mul + add  (per batch column)
                        for bb in range(BLOC):
                            nc.vector.tensor_scalar(
                                filmb[:, m, bb : bb + 1], BC(pfx + "bf", m),
                                mul_sb[:, m, bb : bb + 1], add_sb[:, m, bb : bb + 1],
                                op0=OP.mult, op1=OP.add)

                    # ---- chunk loop over i-rows ------------------------------
                    st1 = psq.tile([128, NCHUNK, 6], F32, tag="st1")
                    denom = psq.tile([8, N], F32, tag="denom")
                    out_raw = psq.tile([128, 2, N], F32, tag="oraw")
                    for c in range(NCHUNK):
                        i0 = c * CH
                        csl = slice(c * CF, (c + 1) * CF)
                        # f = Prelu((gf @ Wf.T) * mul + (bf*mul+add))
                        f_sb = pch.tile([128, 2, CF], F32R, tag="f_sb")
                        for m in range(2):
                            lhs = [LW(f"f_k{k}", m) for k in range(5)]
                            rhs = [
                                edges_t[:, csl],
                                nodes_t[:, 0, :].unsqueeze(1).broadcast_to([128, CH, N]),
                                nodes_t[:, 1, :].unsqueeze(1).broadcast_to([128, CH, N]),
                                nodes_t[:, 0, i0 : i0 + CH].unsqueeze(2).broadcast_to([128, CH, N]),
                                nodes_t[:, 1, i0 : i0 + CH].unsqueeze(2).broadcast_to([128, CH, N]),
                            ]
                            ps = mm_psum(lhs, rhs)
                            nc.scalar.activation(f_sb[:, m, :], ps[:], A.Prelu,
                                                 bias=filmb[:, m, b : b + 1],
                                                 scale=mul_sb[:, m, b : b + 1], alpha=ALPHA)
                        if taps and b == 0 and li == 0 and c == 0:
                            t_ = tap("tap_f0", [128, 2, CF])
                            nc.vector.tensor_copy(tapdbg_f := pch.tile([128, 2, CF], F32, tag="tapf"), f_sb[:])
                            nc.sync.dma_start(t_[:], tapdbg_f[:])
                        # ae = Prelu(f @ We.T + be); x1 = edges + ae; stats
                        ps = mm_psum([LW("e_k0"), LW("e_k1")],
                                     [f_sb[:, 0, :], f_sb[:, 1, :]])
                        nc.scalar.activation(ae_t[:, csl], ps[:], A.Prelu,
                                             bias=BC(pfx + "be"), alpha=ALPHA)
                        nc.vector.tensor_add(ae_t[:, csl], edges_t[:, csl], ae_t[:, csl])
                        nc.vector.bn_stats(st1[:, c, :], ae_t[:, csl])
                        # v mlp
                        vmid = pch.tile([128, 2, CF], F32R, tag="vmid")
                        for m in range(2):
                            ps = mm_psum([LW("v1_k0", m), LW("v1_k1", m)],
                                         [f_sb[:, 0, :], f_sb[:, 1, :]])
                            nc.scalar.activation(vmid[:, m, :], ps[:], A.Prelu,
                                                 bias=BC(pfx + "bv1", m), alpha=ALPHA)
                        v_sb = pch.tile([128, 2, CF], F32, tag="v_sb")
                        for m in range(2):
                            ps = mm_psum([LW("v2_k0", m), LW("v2_k1", m)],
                                         [vmid[:, 0, :], vmid[:, 1, :]])
                            nc.scalar.activation(v_sb[:, m, :], ps[:], A.Identity,
                                                 bias=BC(pfx + "bv2", m))
                        # w mlp -> w^2
                        wmid = pch.tile([128, 2, CF], F32R, tag="wmid")
                        for m in range(2):
                            ps = mm_psum([LW("w1_k0", m), LW("w1_k1", m)],
                                         [f_sb[:, 0, :], f_sb[:, 1, :]])
                            nc.scalar.activation(wmid[:, m, :], ps[:], A.Prelu,
                                                 bias=BC(pfx + "bw1", m), alpha=ALPHA)
                        w2sb = pch.tile([128, 2, CF], F32R, tag="w2sb")
                        for m in range(2):
                            ps = mm_psum([LW("w2_k0", m), LW("w2_k1", m)],
                                         [wmid[:, 0, :], wmid[:, 1, :]])
                            nc.scalar.activation(w2sb[:, m, :], ps[:], A.Square,
                                                 bias=BC(pfx + "bw2", m))
                        # wn2 -> exp(sqrt(q/256))
                        ps_wn = mm_psum([MW("E8_k0"), MW("E8_k1")],
                                        [w2sb[:, 0, :], w2sb[:, 1, :]], (8, CF))
                        expc = pch.tile([8, CF], F32R, tag="expc")
                        nc.scalar.activation(expc[:], ps_wn[:], A.Ln, scale=1.0 / DN)
                        nc.scalar.activation(expc[:], expc[:], A.Exp, scale=0.5)
                        nc.scalar.activation(expc[:], expc[:], A.Exp)
                        nc.vector.tensor_reduce(
                            denom[:, i0 : i0 + CH],
                            expc[:].rearrange("p (i j) -> p i j", i=CH),
                            axis=AX.X, op=OP.add)
                        # attention accumulate: out_raw += sum_j exp * v
                        for m in range(2):
                            ps_x = pp.tile([128, CF], F32, tag="mm")
                            nc.tensor.matmul(ps_x[:], MW("Eexp_k0", m), expc[:],
                                             start=True, stop=True)
                            pv = pch1.tile([128, CF], F32, tag="pv")
                            nc.vector.tensor_mul(pv[:], ps_x[:], v_sb[:, m, :])
                            nc.vector.tensor_reduce(
                                out_raw[:, m, i0 : i0 + CH],
                                pv[:].rearrange("p (i j) -> p i j", i=CH),
                                axis=AX.X, op=OP.add)

                    # ---- attention finalize + node path ----------------------
                    lnden = psm.tile([8, N], F32, tag="lnden")
                    nc.scalar.activation(lnden[:], denom[:], A.Ln)
                    rden = psm.tile([8, N], F32R, tag="rden")
                    nc.scalar.activation(rden[:], lnden[:], A.Exp, scale=-1.0)
                    att = psq.tile([128, 2, N], F32R, tag="att")
                    for m in range(2):
                        ps_r = pp.tile([128, N], F32, tag="mm")
                        nc.tensor.matmul(ps_r[:], MW("Eexp_k0", m), rden[:],
                                         start=True, stop=True)
                        nc.vector.tensor_mul(att[:, m, :], out_raw[:, m, :], ps_r[:])
                    an = psq.tile([128, 2, N], F32, tag="an")
                    for m in range(2):
                        ps = mm_psum([LW("o_k0", m), LW("o_k1", m)],
                                     [att[:, 0, :], att[:, 1, :]], (128, N))
                        nc.scalar.activation(an[:, m, :], ps[:], A.Prelu,
                                             bias=BC(pfx + "bo", m), alpha=ALPHA)
                    if taps and b == 0 and li == 0:
                        t_ = tap("tap_att", [128, 2, N])
                        nc.sync.dma_start(t_[:], an[:])
                        t_ = tap("tap_denom", [8, N])
                        nc.sync.dma_start(t_[:], denom[:])

                    def node_norm(x_t, gname, bname):
                        """Instance-norm x_t [128,2,N] over N per feature -> nodes_t."""
                        for m in range(2):
                            stn = psm.tile([128, 6], F32, tag="stn")
                            nc.vector.bn_stats(stn[:], x_t[:, m, :])
                            mv = psm.tile([128, 2], F32, tag="mvn")
                            nc.vector.bn_aggr(mv[:], stn[:])
                            lnv = psm.tile([128, 1], F32, tag="lnvn")
                            nc.scalar.activation(lnv[:], mv[:, 1:2], A.Ln, bias=BC("eps"))
                            rstd = psm.tile([128, 1], F32, tag="rstdn")
                            nc.scalar.activation(rstd[:], lnv[:], A.Exp, scale=-0.5)
                            s_ = psm.tile([128, 1], F32, tag="sn")
                            nc.vector.tensor_mul(s_[:], rstd[:], BC(gname, m))
                            ms = psm.tile([128, 1], F32, tag="msn")
                            nc.vector.tensor_mul(ms[:], mv[:, 0:1], s_[:])
                            tt_ = psm.tile([128, 1], F32, tag="tn")
                            nc.vector.tensor_sub(tt_[:], BC(bname, m), ms[:])
                            nc.scalar.activation(nodes_t[:, m, :], x_t[:, m, :],
                                                 A.Identity, bias=tt_[:, 0:1],
                                                 scale=s_[:, 0:1])

                    x1n = psq.tile([128, 2, N], F32, tag="x1n")
                    nc.vector.tensor_add(x1n[:], nodes_t[:], an[:])
                    node_norm(x1n, pfx + "n1g", pfx + "n1b")

                    nmid2 = psq.tile([128, 2, N], F32R, tag="nmid2")
                    for m in range(2):
                        ps = mm_psum([LW("mn1_k0", m), LW("mn1_k1", m)],
                                     [nodes_t[:, 0, :], nodes_t[:, 1, :]], (128, N))
                        nc.scalar.activation(nmid2[:, m, :], ps[:], A.Prelu,
                                             bias=BC(pfx + "bmn1", m), alpha=ALPHA)
                    x2n = psq.tile([128, 2, N], F32, tag="x2n")
                    for m in range(2):
                        ps = mm_psum([LW("mn2_k0", m), LW("mn2_k1", m)],
                                     [nmid2[:, 0, :], nmid2[:, 1, :]], (128, N))
                        tmp = psm.tile([128, N], F32, tag="tmpn")
                        nc.scalar.activation(tmp[:], ps[:], A.Prelu,
                                             bias=BC(pfx + "bmn2", m), alpha=ALPHA)
                        nc.vector.tensor_add(x2n[:, m, :], nodes_t[:, m, :], tmp[:])
                    node_norm(x2n, pfx + "n3g", pfx + "n3b")

                    # ---- edge norm n2 (x1 already in ae_t, stats in st1) -----
                    def edge_norm(stats_tile, gname, bname):
                        mv = psm.tile([128, 2], F32, tag="mve")
                        nc.vector.bn_aggr(mv[:], stats_tile[:])
                        lnv = psm.tile([128, 1], F32, tag="lnve")
                        nc.scalar.activation(lnv[:], mv[:, 1:2], A.Ln, bias=BC("eps"))
                        rstd = psm.tile([128, 1], F32, tag="rstde")
                        nc.scalar.activation(rstd[:], lnv[:], A.Exp, scale=-0.5)
                        s_ = psm.tile([128, 1], F32, tag="se")
                        nc.vector.tensor_mul(s_[:], rstd[:], BC(gname))
                        ms = psm.tile([128, 1], F32, tag="mse")
                        nc.vector.tensor_mul(ms[:], mv[:, 0:1], s_[:])
                        tt_ = psm.tile([128, 1], F32, tag="te")
                        nc.vector.tensor_sub(tt_[:], BC(bname), ms[:])
                        nc.scalar.activation(edges_t[:], ae_t[:], A.Identity,
                                             bias=tt_[:, 0:1], scale=s_[:, 0:1])

                    edge_norm(st1, pfx + "n2g", pfx + "n2b")

                    # ---- me mlp + residual + n4 ------------------------------
                    st2 = psq.tile([128, NME, 6], F32, tag="st2")
                    for s in range(NME):
                        sl = slice(s * ME_F, (s + 1) * ME_F)
                        ps = mm_psum([LW("me1_k0")], [edges_t[:, sl]], (128, ME_F))
                        memid = pio.tile([128, ME_F], F32R, tag="pio_r")
                        nc.scalar.activation(memid[:], ps[:], A.Prelu,
                                             bias=BC(pfx + "bme1"), alpha=ALPHA)
                        ps = mm_psum([LW("me2_k0")], [memid[:]], (128, ME_F))
                        meout = pio.tile([128, ME_F], F32, tag="pio_f")
                        nc.scalar.activation(meout[:], ps[:], A.Prelu,
                                             bias=BC(pfx + "bme2"), alpha=ALPHA)
                        nc.vector.tensor_add(ae_t[:, sl], edges_t[:, sl], meout[:])
                        nc.vector.bn_stats(st2[:, s, :], ae_t[:, sl])
                    edge_norm(st2, pfx + "n4g", pfx + "n4b")

                    if taps and b == 0 and li == 0:
                        t_ = tap("tap_nodes_l0", [128, 2, N])
                        nc.sync.dma_start(t_[:], nodes_t[:])
                        t_ = tap("tap_edges_l0", [128, 512])
                        nc.sync.dma_start(t_[:], edges_t[:, 0:512])

                # ================= output heads ===============================
                for w1n, w2n, b1n, b2n, odim, opad, obase in (
                    ("out_t1", "out_t2", "out_t_b1", "out_t_b2", 6, 8, 0),
                    ("out_p1", "out_p2", "out_p_b1", "out_p_b2", 14, 16, 6),
                ):
                    hmid = psq.tile([128, 2, N], F32R, tag="hmid")
                    for m in range(2):
                        ps = mm_psum([MW(f"{w1n}_k0", m), MW(f"{w1n}_k1", m)],
                                     [nodes_t[:, 0, :], nodes_t[:, 1, :]], (128, N))
                        nc.scalar.activation(hmid[:, m, :], ps[:], A.Prelu,
                                             bias=BC(b1n, m), alpha=ALPHA)
                    ps = mm_psum([MW(f"{w2n}_k0"), MW(f"{w2n}_k1")],
                                 [hmid[:, 0, :], hmid[:, 1, :]], (opad, N))
                    no_sb = psq.tile([14, N], F32, tag="nosb" + w1n)
                    nc.scalar.activation(no_sb[:odim, :], ps[:odim, :],
                                         A.Identity, bias=BC(b2n, h=odim))
                    nc.sync.dma_start(d_on[b][obase : obase + odim, :], no_sb[:odim, :])

                for s in range(NME):
                    sl = slice(s * ME_F, (s + 1) * ME_F)
                    ps = mm_psum([MW("out_e1_k0")], [edges_t[:, sl]], (128, ME_F))
                    oemid = pio.tile([128, ME_F], F32R, tag="pio_r")
                    nc.scalar.activation(oemid[:], ps[:], A.Prelu,
                                         bias=BC("out_e_b1"), alpha=ALPHA)
                    ps = mm_psum([MW("out_e2_k0")], [oemid[:]], (8, ME_F))
                    oe_sb = pio.tile([EDGE_D, ME_F], F32, tag="pio_o")
                    nc.scalar.activation(oe_sb[:], ps[:EDGE_D, :], A.Identity,
                                         bias=BC("out_e_b2", h=EDGE_D))
                    nc.sync.dma_start(d_oe[b][:, sl], oe_sb[:])

    nc.compile()
    return nc


_PROG_CACHE = {}


def _get_program(misc_off, misc_cols, la_off, la_cols, lb_off, lb_cols, bc_off, bc_cols, taps=False):
    key = (misc_cols, la_cols, lb_cols, bc_cols, taps)
    if key not in _PROG_CACHE:
        _PROG_CACHE[key] = _build_program(
            misc_off, misc_cols, la_off, la_cols, lb_off, lb_cols, bc_off, bc_cols, taps=taps)
    return _PROG_CACHE[key]


def kernel(nodes, edges, timestep, params, _taps=False, _trace=False):
    global LAST_RESULTS
    nodes = np.asarray(nodes, np.float32)
    edges = np.asarray(edges, np.float32)
    timestep = np.asarray(timestep)

    misc, layer_wa, laya, layer_wb, layb, bc = _pack_host(params)
    misc_blob = misc.blob()
    bc_blob = bc.blob()

    nc = _get_program(misc.off, misc_blob.shape[1], laya.off, layer_wa.shape[2],
                      layb.off, layer_wb.shape[2], bc.off, bc_blob.shape[1], taps=_taps)

    nodes_fm = np.ascontiguousarray(nodes.transpose(0, 2, 1))          # [B,20,96]
    edges_fm = np.zeros((B, 8, NN), np.float32)
    edges_fm[:, :EDGE_D, :] = edges.reshape(B, NN, EDGE_D).transpose(0, 2, 1)
    t_f32 = timestep.astype(np.float32)

    in_maps = []
    for c in range(NCORES):
        bs = slice(c * BLOC, (c + 1) * BLOC)
        in_maps.append({
            "misc_w": misc_blob,
            "layer_wa": layer_wa,
            "layer_wb": layer_wb,
            "bias_c": bc_blob,
            "nodes_fm": nodes_fm[bs],
            "edges_fm": edges_fm[bs],
            "t_f32": np.broadcast_to(t_f32[bs], (4, BLOC)).copy(),
        })

    res = bass_utils.run_bass_kernel_spmd(
        nc, in_maps, core_ids=list(range(NCORES)), trace=_trace)
    LAST_RESULTS = res

    on = np.concatenate([r["out_nodes_fm"] for r in res.results])      # [B,20,96]
    oe = np.concatenate([r["out_edges_fm"] for r in res.results])      # [B,5,9216]
    out_nodes = np.ascontiguousarray(on.transpose(0, 2, 1))            # [B,96,20]
    out_edges = np.ascontiguousarray(
        oe.transpose(0, 2, 1).reshape(B, N, N, EDGE_D))
    return out_nodes, out_edges


# revision 6
# speedup vs baseline: 1.2163x; 1.2163x over previous
# Trainium2 Bass kernel for nn_DiffusionModel_88948772700534 (gnn_message_passing).
#
# Strategy: data-parallel over batch B=16 across 8 NeuronCores (2 batches/core).
# All activations are kept feature-major on-chip ([feature_partition, token]) so
# every Linear becomes PE matmuls accumulated over K in PSUM, with bias +
# LeakyReLU (Prelu) fused into a single ScalarE activation reading PSUM.
# Matmuls run in float32r (full-rate fp32); weights are streamed per layer from
# HBM. The pairwise (i,j) tensors are processed in chunks of 4 i-rows (384
# pairs) so nothing pairwise is ever materialized in HBM; gf (the 640-dim
# concat) is never materialized at all - the f-matmul reads the edge state
# plus broadcast APs of the node state directly.
#
# sqrt and 1/x are computed as exp(0.5*ln(x)) / exp(-ln(x)) so that the whole
# kernel stays inside one ScalarE activation table set
# (natural_log_exp_and_others: exp/ln/square/identity/parametric_relu),
# avoiding ~2.7us table reloads. Softmax uses unnormalized exp (wn >= 0 is
# small) with the denominator folded in after the attention reduction.

import os
import sys
import types

for _p in ("/opt/trn_rl_repo",):
    if _p not in sys.path:
        sys.path.insert(0, _p)

# NTFF profile hook (lets BASS_TRACE=1 capture HW timing under axon).
try:
    import antenv

    if "antenv.axon_hooks" not in sys.modules:
        _hooks = types.ModuleType("antenv.axon_hooks")
        _hook_slot = [None]
        _hooks.set_axon_ntff_profile_hook = lambda h: _hook_slot.__setitem__(0, h)
        _hooks.get_axon_ntff_profile_hook = lambda: _hook_slot[0]
        sys.modules["antenv.axon_hooks"] = _hooks
        antenv.axon_hooks = _hooks
        try:
            from trn_agent_boot.trn_boot import _ntff_profile_via_ctypes

            _h = _ntff_profile_via_ctypes("/opt/axon/libaxon_pjrt.so")
            if _h is not None:
                _hooks.set_axon_ntff_profile_hook(_h)
        except Exception:
            pass
except Exception:
    pass

import numpy as np

import concourse.bass as bass
import concourse.tile as tile
from concourse import bacc, bass_utils, mybir

F32 = mybir.dt.float32
F32R = mybir.dt.float32r
I32 = mybir.dt.int32
A = mybir.ActivationFunctionType
OP = mybir.AluOpType
AX = mybir.AxisListType

B, N = 16, 96
NODE_D, EDGE_D = 20, 5
DN, DE, DC = 256, 128, 256
H, L, TMAX = 8, 4, 1000
EPS = 1e-5
ALPHA = 0.01
NCORES = 8
BLOC = B // NCORES          # batches per core
NN = N * N                  # 9216 pairs
CH = 4                      # i-rows per chunk
CF = CH * N                 # chunk free size (384)
NCHUNK = N // CH            # 24
ME_F = 512                  # free size for 128-feature edge passes
NME = NN // ME_F            # 18

LAST_RESULTS = None         # filled by kernel() for inspection (exec_time_ns etc.)


class _Packer:
    """Packs 2D slabs [h<=128, w] into one [128, cols] blob, column-major."""

    def __init__(self):
        self.cols = 0
        self.off = {}
        self.parts = []

    def add(self, name, arr):
        arr = np.asarray(arr, np.float32)
        assert arr.ndim == 2 and arr.shape[0] <= 128
        self.off[name] = (self.cols, arr.shape[1], arr.shape[0])
        self.parts.append((self.cols, arr))
        self.cols += arr.shape[1]

    def blob(self, cols=None):
        out = np.zeros((128, cols or self.cols), np.float32)
        for c, arr in self.parts:
            out[: arr.shape[0], c : c + arr.shape[1]] = arr
        return out


def _add_lin(pk, name, w):
    """Linear weight w [O, I] -> k-slabs of w.T, named name_k{k}."""
    wT = np.asarray(w, np.float32).T  # [I, O]
    I_, O = wT.shape
    nk = (I_ + 127) // 128
    for k in range(nk):
        pk.add(f"{name}_k{k}", wT[k * 128 : (k + 1) * 128, :])


def _add_bias(pk, name, v):
    """Bias/norm vector [O] -> one column per 128-block, named name (width nm)."""
    v = np.asarray(v, np.float32).reshape(-1)
    O = v.shape[0]
    nm = (O + 127) // 128
    cols = np.zeros((128, nm), np.float32)
    for m in range(nm):
        seg = v[m * 128 : (m + 1) * 128]
        cols[: seg.shape[0], m] = seg
    pk.add(name, cols)


def _pack_host(params):
    """Build the three weight blobs + offset maps from the params pytree."""
    p = {k: params[k] for k in params}

    misc = _Packer()
    _add_lin(misc, "in_n1", p["in_n"][0])
    _add_lin(misc, "in_n2", p["in_n"][2])
    w_ie1 = np.zeros((128, 8), np.float32)
    w_ie1[:, :EDGE_D] = np.asarray(p["in_e"][0], np.float32)
    _add_lin(misc, "in_e1", w_ie1)
    _add_lin(misc, "in_e2", p["in_e"][2])
    _add_lin(misc, "in_c1", p["in_c"][0])
    _add_lin(misc, "in_c2", p["in_c"][2])
    _add_lin(misc, "out_t1", p["out_t"][0])
    w_ot2 = np.zeros((8, DN), np.float32)
    w_ot2[:6] = np.asarray(p["out_t"][2], np.float32)
    _add_lin(misc, "out_t2", w_ot2)
    _add_lin(misc, "out_p1", p["out_p"][0])
    w_op2 = np.zeros((16, DN), np.float32)
    w_op2[:14] = np.asarray(p["out_p"][2], np.float32)
    _add_lin(misc, "out_p2", w_op2)
    _add_lin(misc, "out_e1", p["out_e"][0])
    w_oe2 = np.zeros((8, DE), np.float32)
    w_oe2[:EDGE_D] = np.asarray(p["out_e"][2], np.float32)
    _add_lin(misc, "out_e2", w_oe2)
    # E8: [256, 8] block-diag ones (sum w^2 within each head) -> 2 k-slabs
    e8 = np.zeros((256, 8), np.float32)
    for hd in range(256):
        e8[hd, hd // 32] = 1.0
    _add_lin(misc, "E8", e8.T)          # treat as w [8, 256]
    # Eexp: lhsT [8, 256] (broadcast head value to its 32 dims)
    _add_lin(misc, "Eexp", e8)          # w [256, 8] -> wT [8, 256] single slab
    misc.add("ones1", np.full((4, 128), 0.25, np.float32))

    laya = layb = None
    a_blobs, b_blobs = [], []
    for li in range(L):
        lp = p["layers"][li]
        mha = lp["mha"]
        pa = _Packer()
        _add_lin(pa, "f", mha["f"][0])
        _add_lin(pa, "e", mha["e"][0])
        _add_lin(pa, "v1", mha["v"][0])
        _add_lin(pa, "v2", mha["v"][2])
        _add_lin(pa, "w1", mha["w"][0])
        _add_lin(pa, "w2", mha["w"][2])
        _add_lin(pa, "m", mha["m"][0])
        _add_lin(pa, "a", mha["a"][0])
        pb2 = _Packer()
        _add_lin(pb2, "o", mha["o"][0])
        _add_lin(pb2, "mn1", lp["mn"][0])
        _add_lin(pb2, "mn2", lp["mn"][2])
        _add_lin(pb2, "me1", lp["me"][0])
        _add_lin(pb2, "me2", lp["me"][2])
        if laya is None:
            laya, layb = pa, pb2
        a_blobs.append(pa.blob())
        b_blobs.append(pb2.blob())
    layer_wa = np.stack(a_blobs)
    layer_wb = np.stack(b_blobs)

    bc = _Packer()
    # time-embedding constants: s/(2pi) and phase (0 or 0.25), packed per m-col
    scales = np.exp(np.arange(0, DC, 2, dtype=np.float64) * (-np.log(10000.0) / DC))
    sp = np.repeat(scales, 2) / (2.0 * np.pi)   # [256]
    ph = np.tile([0.0, 0.25], DC // 2)          # [256]
    _add_bias(bc, "eps", np.full(128, EPS))
    _add_bias(bc, "s2pi", sp)
    _add_bias(bc, "phase", ph)
    _add_bias(bc, "in_n_b1", p["in_n"][1])
    _add_bias(bc, "in_n_b2", p["in_n"][3])
    _add_bias(bc, "in_e_b1", p["in_e"][1])
    _add_bias(bc, "in_e_b2", p["in_e"][3])
    _add_bias(bc, "in_c_b1", p["in_c"][1])
    _add_bias(bc, "in_c_b2", p["in_c"][3])
    _add_bias(bc, "out_t_b1", p["out_t"][1])
    _add_bias(bc, "out_t_b2", p["out_t"][3])
    _add_bias(bc, "out_p_b1", p["out_p"][1])
    _add_bias(bc, "out_p_b2", p["out_p"][3])
    _add_bias(bc, "out_e_b1", p["out_e"][1])
    _add_bias(bc, "out_e_b2", p["out_e"][3])
    for li in range(L):
        lp = p["layers"][li]
        mha = lp["mha"]
        _add_bias(bc, f"l{li}_bf", mha["f"][1])
        _add_bias(bc, f"l{li}_be", mha["e"][1])
        _add_bias(bc, f"l{li}_bv1", mha["v"][1])
        _add_bias(bc, f"l{li}_bv2", mha["v"][3])
        _add_bias(bc, f"l{li}_bw1", mha["w"][1])
        _add_bias(bc, f"l{li}_bw2", mha["w"][3])
        _add_bias(bc, f"l{li}_bo", mha["o"][1])
        _add_bias(bc, f"l{li}_bm", mha["m"][1])
        _add_bias(bc, f"l{li}_ba", mha["a"][1])
        _add_bias(bc, f"l{li}_bmn1", lp["mn"][1])
        _add_bias(bc, f"l{li}_bmn2", lp["mn"][3])
        _add_bias(bc, f"l{li}_bme1", lp["me"][1])
        _add_bias(bc, f"l{li}_bme2", lp["me"][3])
        for nn_, wdt in (("n1", DN), ("n2", DE), ("n3", DN), ("n4", DE)):
            g, b_ = lp[nn_]
            _add_bias(bc, f"l{li}_{nn_}g", g)
            _add_bias(bc, f"l{li}_{nn_}b", b_)

    return misc, layer_wa, laya, layer_wb, layb, bc


def _build_program(misc_off, misc_cols, la_off, la_cols, lb_off, lb_cols, bc_off, bc_cols, taps=False):
    nc = bacc.Bacc("TRN2", debug=False)

    d_misc = nc.dram_tensor("misc_w", [128, misc_cols], F32R, kind="ExternalInput").ap()
    d_laya = nc.dram_tensor("layer_wa", [L, 128, la_cols], F32R, kind="ExternalInput").ap()
    d_layb = nc.dram_tensor("layer_wb", [L, 128, lb_cols], F32R, kind="ExternalInput").ap()
    d_bc = nc.dram_tensor("bias_c", [128, bc_cols], F32, kind="ExternalInput").ap()
    d_nodes = nc.dram_tensor("nodes_fm", [BLOC, NODE_D, N], F32R, kind="ExternalInput").ap()
    d_edges = nc.dram_tensor("edges_fm", [BLOC, 8, NN], F32R, kind="ExternalInput").ap()
    d_t = nc.dram_tensor("t_f32", [4, BLOC], F32R, kind="ExternalInput").ap()
    d_on = nc.dram_tensor("out_nodes_fm", [BLOC, NODE_D, N], F32, kind="ExternalOutput").ap()
    d_oe = nc.dram_tensor("out_edges_fm", [BLOC, EDGE_D, NN], F32, kind="ExternalOutput").ap()

    tapd = {}

    def tap(name, shape):
        if taps:
            tapd[name] = nc.dram_tensor(name, shape, F32, kind="ExternalOutput").ap()
        return tapd.get(name)

    with tile.TileContext(nc) as tc:
        with (
            tc.tile_pool(name="pw_misc", bufs=1) as pw_misc,
            tc.tile_pool(name="pw_laya", bufs=2) as pw_laya,
            tc.tile_pool(name="pw_layb", bufs=1) as pw_layb,
            tc.tile_pool(name="pb", bufs=1) as pbp,
            tc.tile_pool(name="pstate", bufs=1) as pstate,
            tc.tile_pool(name="pchunk", bufs=2) as pch,
            tc.tile_pool(name="pchunk1", bufs=1) as pch1,
            tc.tile_pool(name="psmall", bufs=2) as psm,
            tc.tile_pool(name="pseq", bufs=1) as psq,
            tc.tile_pool(name="pio", bufs=2) as pio,
            tc.tile_pool(name="pp", bufs=8, space="PSUM") as pp,
        ):
            mw = pw_misc.tile([128, misc_cols], F32R, tag="misc")
            nc.sync.dma_start(mw[:], d_misc[:])
            bcw = pbp.tile([128, bc_cols], F32, tag="bias")
            nc.sync.dma_start(bcw[:], d_bc[:])

            def MW(name, m=0, mw_width=128):
                off, w, h = misc_off[name]
                return mw[:h, off + m * mw_width : off + min((m + 1) * mw_width, w)]

            def BC(name, m=0, h=128):
                off, w, _ = bc_off[name]
                return bcw[:h, off + m : off + m + 1]

            def mm_psum(lhs_list, rhs_list, out_shape=(128, CF)):
                """Accumulating matmul chain into a fresh psum tile."""
                ps = pp.tile(list(out_shape), F32, tag="mm")
                nk = len(lhs_list)
                for k in range(nk):
                    nc.tensor.matmul(
                        ps[:], lhs_list[k], rhs_list[k],
                        start=(k == 0), stop=(k == nk - 1),
                    )
                return ps

            # ============ per-core program: loop over local batches ============
            for b in range(BLOC):
                # ---- persistent per-batch state tiles (reused across batches) --
                edges_t = pstate.tile([128, NN], F32R, tag="edges")
                ae_t = pstate.tile([128, NN], F32, tag="ae")
                nodes_t = psq.tile([128, 2, N], F32R, tag="nodes")

                # ---------------- time embedding + cond -----------------------
                t_sb = psm.tile([4, BLOC], F32R, tag="tsb")
                nc.sync.dma_start(t_sb[:], d_t[:])
                ps_t = pp.tile([128, BLOC], F32, tag="mm")
                nc.tensor.matmul(ps_t[:], MW("ones1"), t_sb[:], start=True, stop=True)
                emb = psq.tile([128, 2, BLOC], F32R, tag="emb")
                for m in range(2):
                    y = psm.tile([128, BLOC], F32, tag="y")
                    nc.scalar.activation(y[:], ps_t[:], A.Identity,
                                         bias=BC("phase", m), scale=BC("s2pi", m))
                    ki = psm.tile([128, BLOC], I32, tag="ki")
                    nc.vector.tensor_copy(ki[:], y[:])
                    kf = psm.tile([128, BLOC], F32, tag="kf")
                    nc.vector.tensor_copy(kf[:], ki[:])
                    fr = psm.tile([128, BLOC], F32, tag="fr")
                    nc.vector.tensor_sub(fr[:], y[:], kf[:])
                    nc.scalar.activation(emb[:, m, :], fr[:], A.Sin, scale=float(2 * np.pi))
                cmid = psq.tile([128, 2, BLOC], F32R, tag="cmid")
                for m in range(2):
                    ps = mm_psum([MW("in_c1_k0", m), MW("in_c1_k1", m)],
                                 [emb[:, 0, :], emb[:, 1, :]], (128, BLOC))
                    nc.scalar.activation(cmid[:, m, :], ps[:], A.Prelu,
                                         bias=BC("in_c_b1", m), alpha=ALPHA)
                cond = psq.tile([128, 2, BLOC], F32R, tag="cond")
                for m in range(2):
                    ps = mm_psum([MW("in_c2_k0", m), MW("in_c2_k1", m)],
                                 [cmid[:, 0, :], cmid[:, 1, :]], (128, BLOC))
                    nc.scalar.activation(cond[:, m, :], ps[:], A.Prelu,
                                         bias=BC("in_c_b2", m), alpha=ALPHA)
                if b == 0:
                    t_ = tap("tap_cond", [128, 2, BLOC])
                    if t_ is not None:
                        nc.sync.dma_start(t_[:], cond[:])

                # ---------------- input MLPs ---------------------------------
                nraw = psq.tile([NODE_D, N], F32R, tag="nraw")
                nc.sync.dma_start(nraw[:], d_nodes[b])
                nmid = psq.tile([128, 2, N], F32R, tag="nmid")
                for m in range(2):
                    ps = mm_psum([MW("in_n1_k0", m)], [nraw[:]], (128, N))
                    nc.scalar.activation(nmid[:, m, :], ps[:], A.Prelu,
                                         bias=BC("in_n_b1", m), alpha=ALPHA)
                for m in range(2):
                    ps = mm_psum([MW("in_n2_k0", m), MW("in_n2_k1", m)],
                                 [nmid[:, 0, :], nmid[:, 1, :]], (128, N))
                    nc.scalar.activation(nodes_t[:, m, :], ps[:], A.Prelu,
                                         bias=BC("in_n_b2", m), alpha=ALPHA)

                for s in range(NME):
                    sl = slice(s * ME_F, (s + 1) * ME_F)
                    eraw = pio.tile([8, ME_F], F32R, tag="pio_e")
                    nc.sync.dma_start(eraw[:], d_edges[b][:, sl])
                    ps = mm_psum([MW("in_e1_k0")], [eraw[:]], (128, ME_F))
                    emid = pio.tile([128, ME_F], F32R, tag="pio_r")
                    nc.scalar.activation(emid[:], ps[:], A.Prelu,
                                         bias=BC("in_e_b1"), alpha=ALPHA)
                    ps = mm_psum([MW("in_e2_k0")], [emid[:]], (128, ME_F))
                    nc.scalar.activation(edges_t[:, sl], ps[:], A.Prelu,
                                         bias=BC("in_e_b2"), alpha=ALPHA)

                if b == 0:
                    t_ = tap("tap_nodes_in", [128, 2, N])
                    if t_ is not None:
                        nc.sync.dma_start(t_[:], nodes_t[:])
                    t_ = tap("tap_edges_in", [128, 512])
                    if t_ is not None:
                        nc.sync.dma_start(t_[:], edges_t[:, 0:512])

                # ================= layers =====================================
                for li in range(L):
                    lwa = pw_laya.tile([128, la_cols], F32R, tag="laywa")
                    nc.sync.dma_start(lwa[:], d_laya[li])
                    lwb = pw_layb.tile([128, lb_cols], F32R, tag="laywb")
                    nc.sync.dma_start(lwb[:], d_layb[li])

                    def LW(name, m=0, _la=lwa, _lb=lwb):
                        if name.split("_k")[0] in ("o", "mn1", "mn2", "me1", "me2"):
                            off, w, h = lb_off[name]
                            return _lb[:h, off + m * 128 : off + min((m + 1) * 128, w)]
                        off, w, h = la_off[name]
                        return _la[:h, off + m * 128 : off + min((m + 1) * 128, w)]

                    pfx = f"l{li}_"

                    # ---- FiLM params: mul/add = cond @ Wm/Wa + b -------------
                    mul_sb = psm.tile([128, 2, BLOC], F32, tag="mul")
                    filmb = psm.tile([128, 2, BLOC], F32, tag="filmb")
                    add_sb = psm.tile([128, 2, BLOC], F32, tag="adds")
                    for m in range(2):
                        ps = mm_psum([LW("m_k0", m), LW("m_k1", m)],
                                     [cond[:, 0, :], cond[:, 1, :]], (128, BLOC))
                        nc.scalar.activation(mul_sb[:, m, :], ps[:], A.Identity,
                                             bias=BC(pfx + "bm", m))
                        ps = mm_psum([LW("a_k0", m), LW("a_k1", m)],
                                     [cond[:, 0, :], cond[:, 1, :]], (128, BLOC))
                        nc.scalar.activation(add_sb[:, m, :], ps[:], A.Identity,
                                             bias=BC(pfx + "ba", m))
                        # filmb = bf * mul + add  (per batch column)
                        for bb in range(BLOC):
                            nc.vector.tensor_scalar(
                                filmb[:, m, bb : bb + 1], BC(pfx + "bf", m),
                                mul_sb[:, m, bb : bb + 1], add_sb[:, m, bb : bb + 1],
                                op0=OP.mult, op1=OP.add)

                    # ---- chunk loop over i-rows ------------------------------
                    st1 = psq.tile([128, NCHUNK, 6], F32, tag="st1")
                    denom = psq.tile([8, N], F32, tag="denom")
                    out_raw = psq.tile([128, 2, N], F32, tag="oraw")
                    for c in range(NCHUNK):
                        i0 = c * CH
                        csl = slice(c * CF, (c + 1) * CF)
                        # f = Prelu((gf @ Wf.T) * mul + (bf*mul+add))
                        f_sb = pch.tile([128, 2, CF], F32R, tag="f_sb")
                        for m in range(2):
                            lhs = [LW(f"f_k{k}", m) for k in range(5)]
                            rhs = [
                                edges_t[:, csl],
                                nodes_t[:, 0, :].unsqueeze(1).broadcast_to([128, CH, N]),
                                nodes_t[:, 1, :].unsqueeze(1).broadcast_to([128, CH, N]),
                                nodes_t[:, 0, i0 : i0 + CH].unsqueeze(2).broadcast_to([128, CH, N]),
                                nodes_t[:, 1, i0 : i0 + CH].unsqueeze(2).broadcast_to([128, CH, N]),
                            ]
                            ps = mm_psum(lhs, rhs)
                            nc.scalar.activation(f_sb[:, m, :], ps[:], A.Prelu,
                                                 bias=filmb[:, m, b : b + 1],
                                                 scale=mul_sb[:, m, b : b + 1], alpha=ALPHA)
                        if taps and b == 0 and li == 0 and c == 0:
                            t_ = tap("tap_f0", [128, 2, CF])
                            nc.vector.tensor_copy(tapdbg_f := pch.tile([128, 2, CF], F32, tag="tapf"), f_sb[:])
                            nc.sync.dma_start(t_[:], tapdbg_f[:])
                        # ae = Prelu(f @ We.T + be); x1 = edges + ae; stats
                        ps = mm_psum([LW("e_k0"), LW("e_k1")],
                                     [f_sb[:, 0, :], f_sb[:, 1, :]])
                        nc.scalar.activation(ae_t[:, csl], ps[:], A.Prelu,
                                             bias=BC(pfx + "be"), alpha=ALPHA)
                        nc.vector.tensor_add(ae_t[:, csl], edges_t[:, csl], ae_t[:, csl])
                        nc.vector.bn_stats(st1[:, c, :], ae_t[:, csl])
                        # v mlp
                        vmid = pch.tile([128, 2, CF], F32R, tag="vmid")
                        for m in range(2):
                            ps = mm_psum([LW("v1_k0", m), LW("v1_k1", m)],
                                         [f_sb[:, 0, :], f_sb[:, 1, :]])
                            nc.scalar.activation(vmid[:, m, :], ps[:], A.Prelu,
                                                 bias=BC(pfx + "bv1", m), alpha=ALPHA)
                        v_sb = pch.tile([128, 2, CF], F32, tag="v_sb")
                        for m in range(2):
                            ps = mm_psum([LW("v2_k0", m), LW("v2_k1", m)],
                                         [vmid[:, 0, :], vmid[:, 1, :]])
                            nc.vector.tensor_scalar(v_sb[:, m, :], ps[:],
                                                    BC(pfx + "bv2", m), 0.0,
                                                    op0=OP.add, op1=OP.add)
                        # w mlp -> w^2
                        wmid = pch.tile([128, 2, CF], F32R, tag="wmid")
                        for m in range(2):
                            ps = mm_psum([LW("w1_k0", m), LW("w1_k1", m)],
                                         [f_sb[:, 0, :], f_sb[:, 1, :]])
                            nc.scalar.activation(wmid[:, m, :], ps[:], A.Prelu,
                                                 bias=BC(pfx + "bw1", m), alpha=ALPHA)
                        w2sb = pch.tile([128, 2, CF], F32R, tag="w2sb")
                        for m in range(2):
                            ps = mm_psum([LW("w2_k0", m), LW("w2_k1", m)],
                                         [wmid[:, 0, :], wmid[:, 1, :]])
                            nc.scalar.activation(w2sb[:, m, :], ps[:], A.Square,
                                                 bias=BC(pfx + "bw2", m))
                        # wn2 -> exp(sqrt(q/256))
                        ps_wn = mm_psum([MW("E8_k0"), MW("E8_k1")],
                                        [w2sb[:, 0, :], w2sb[:, 1, :]], (8, CF))
                        expc = pch.tile([8, CF], F32R, tag="expc")
                        nc.scalar.activation(expc[:], ps_wn[:], A.Ln, scale=1.0 / DN)
                        nc.scalar.activation(expc[:], expc[:], A.Exp, scale=0.5)
                        nc.scalar.activation(expc[:], expc[:], A.Exp)
                        nc.vector.tensor_reduce(
                            denom[:, i0 : i0 + CH],
                            expc[:].rearrange("p (i j) -> p i j", i=CH),
                            axis=AX.X, op=OP.add)
                        # attention accumulate: out_raw += sum_j exp * v
                        for m in range(2):
                            ps_x = pp.tile([128, CF], F32, tag="mm")
                            nc.tensor.matmul(ps_x[:], MW("Eexp_k0", m), expc[:],
                                             start=True, stop=True)
                            pv = pch1.tile([128, CF], F32, tag="pv")
                            nc.vector.tensor_mul(pv[:], ps_x[:], v_sb[:, m, :])
                            nc.vector.tensor_reduce(
                                out_raw[:, m, i0 : i0 + CH],
                                pv[:].rearrange("p (i j) -> p i j", i=CH),
                                axis=AX.X, op=OP.add)

                    # ---- attention finalize + node path ----------------------
                    lnden = psm.tile([8, N], F32, tag="lnden")
                    nc.scalar.activation(lnden[:], denom[:], A.Ln)
                    rden = psm.tile([8, N], F32R, tag="rden")
                    nc.scalar.activation(rden[:], lnden[:], A.Exp, scale=-1.0)
                    att = psq.tile([128, 2, N], F32R, tag="att")
                    for m in range(2):
                        ps_r = pp.tile([128, N], F32, tag="mm")
                        nc.tensor.matmul(ps_r[:], MW("Eexp_k0", m), rden[:],
                                         start=True, stop=True)
                        nc.vector.tensor_mul(att[:, m, :], out_raw[:, m, :], ps_r[:])
                    an = psq.tile([128, 2, N], F32, tag="an")
                    for m in range(2):
                        ps = mm_psum([LW("o_k0", m), LW("o_k1", m)],
                                     [att[:, 0, :], att[:, 1, :]], (128, N))
                        nc.scalar.activation(an[:, m, :], ps[:], A.Prelu,
                                             bias=BC(pfx + "bo", m), alpha=ALPHA)
                    if taps and b == 0 and li == 0:
                        t_ = tap("tap_att", [128, 2, N])
                        nc.sync.dma_start(t_[:], an[:])
                        t_ = tap("tap_denom", [8, N])
                        nc.sync.dma_start(t_[:], denom[:])

                    def node_norm(x_t, gname, bname):
                        """Instance-norm x_t [128,2,N] over N per feature -> nodes_t."""
                        for m in range(2):
                            stn = psm.tile([128, 6], F32, tag="stn")
                            nc.vector.bn_stats(stn[:], x_t[:, m, :])
                            mv = psm.tile([128, 2], F32, tag="mvn")
                            nc.vector.bn_aggr(mv[:], stn[:])
                            lnv = psm.tile([128, 1], F32, tag="lnvn")
                            nc.scalar.activation(lnv[:], mv[:, 1:2], A.Ln, bias=BC("eps"))
                            rstd = psm.tile([128, 1], F32, tag="rstdn")
                            nc.scalar.activation(rstd[:], lnv[:], A.Exp, scale=-0.5)
                            s_ = psm.tile([128, 1], F32, tag="sn")
                            nc.vector.tensor_mul(s_[:], rstd[:], BC(gname, m))
                            ms = psm.tile([128, 1], F32, tag="msn")
                            nc.vector.tensor_mul(ms[:], mv[:, 0:1], s_[:])
                            tt_ = psm.tile([128, 1], F32, tag="tn")
                            nc.vector.tensor_sub(tt_[:], BC(bname, m), ms[:])
                            nc.scalar.activation(nodes_t[:, m, :], x_t[:, m, :],
                                                 A.Identity, bias=tt_[:, 0:1],
                                                 scale=s_[:, 0:1])

                    x1n = psq.tile([128, 2, N], F32, tag="x1n")
                    nc.vector.tensor_add(x1n[:], nodes_t[:], an[:])
                    node_norm(x1n, pfx + "n1g", pfx + "n1b")

                    nmid2 = psq.tile([128, 2, N], F32R, tag="nmid2")
                    for m in range(2):
                        ps = mm_psum([LW("mn1_k0", m), LW("mn1_k1", m)],
                                     [nodes_t[:, 0, :], nodes_t[:, 1, :]], (128, N))
                        nc.scalar.activation(nmid2[:, m, :], ps[:], A.Prelu,
                                             bias=BC(pfx + "bmn1", m), alpha=ALPHA)
                    x2n = psq.tile([128, 2, N], F32, tag="x2n")
                    for m in range(2):
                        ps = mm_psum([LW("mn2_k0", m), LW("mn2_k1", m)],
                                     [nmid2[:, 0, :], nmid2[:, 1, :]], (128, N))
                        tmp = psm.tile([128, N], F32, tag="tmpn")
                        nc.scalar.activation(tmp[:], ps[:], A.Prelu,
                                             bias=BC(pfx + "bmn2", m), alpha=ALPHA)
                        nc.vector.tensor_add(x2n[:, m, :], nodes_t[:, m, :], tmp[:])
                    node_norm(x2n, pfx + "n3g", pfx + "n3b")

                    # ---- edge norm n2 (x1 already in ae_t, stats in st1) -----
                    def edge_norm(stats_tile, gname, bname):
                        mv = psm.tile([128, 2], F32, tag="mve")
                        nc.vector.bn_aggr(mv[:], stats_tile[:])
                        lnv = psm.tile([128, 1], F32, tag="lnve")
                        nc.scalar.activation(lnv[:], mv[:, 1:2], A.Ln, bias=BC("eps"))
                        rstd = psm.tile([128, 1], F32, tag="rstde")
                        nc.scalar.activation(rstd[:], lnv[:], A.Exp, scale=-0.5)
                        s_ = psm.tile([128, 1], F32, tag="se")
                        nc.vector.tensor_mul(s_[:], rstd[:], BC(gname))
                        ms = psm.tile([128, 1], F32, tag="mse")
                        nc.vector.tensor_mul(ms[:], mv[:, 0:1], s_[:])
                        tt_ = psm.tile([128, 1], F32, tag="te")
                        nc.vector.tensor_sub(tt_[:], BC(bname), ms[:])
                        nc.scalar.activation(edges_t[:], ae_t[:], A.Identity,
                                             bias=tt_[:, 0:1], scale=s_[:, 0:1])

                    edge_norm(st1, pfx + "n2g", pfx + "n2b")

                    # ---- me mlp + residual + n4 ------------------------------
                    st2 = psq.tile([128, NME, 6], F32, tag="st2")
                    for s in range(NME):
                        sl = slice(s * ME_F, (s + 1) * ME_F)
                        ps = mm_psum([LW("me1_k0")], [edges_t[:, sl]], (128, ME_F))
                        memid = pio.tile([128, ME_F], F32R, tag="pio_r")
                        nc.scalar.activation(memid[:], ps[:], A.Prelu,
                                             bias=BC(pfx + "bme1"), alpha=ALPHA)
                        ps = mm_psum([LW("me2_k0")], [memid[:]], (128, ME_F))
                        meout = pio.tile([128, ME_F], F32, tag="pio_f")
                        nc.scalar.activation(meout[:], ps[:], A.Prelu,
                                             bias=BC(pfx + "bme2"), alpha=ALPHA)
                        nc.vector.tensor_add(ae_t[:, sl], edges_t[:, sl], meout[:])
                        nc.vector.bn_stats(st2[:, s, :], ae_t[:, sl])
                    edge_norm(st2, pfx + "n4g", pfx + "n4b")

                    if taps and b == 0 and li == 0:
                        t_ = tap("tap_nodes_l0", [128, 2, N])
                        nc.sync.dma_start(t_[:], nodes_t[:])
                        t_ = tap("tap_edges_l0", [128, 512])
                        nc.sync.dma_start(t_[:], edges_t[:, 0:512])

                # ================= output heads ===============================
                for w1n, w2n, b1n, b2n, odim, opad, obase in (
                    ("out_t1", "out_t2", "out_t_b1", "out_t_b2", 6, 8, 0),
                    ("out_p1", "out_p2", "out_p_b1", "out_p_b2", 14, 16, 6),
                ):
                    hmid = psq.tile([128, 2, N], F32R, tag="hmid")
                    for m in range(2):
                        ps = mm_psum([MW(f"{w1n}_k0", m), MW(f"{w1n}_k1", m)],
                                     [nodes_t[:, 0, :], nodes_t[:, 1, :]], (128, N))
                        nc.scalar.activation(hmid[:, m, :], ps[:], A.Prelu,
                                             bias=BC(b1n, m), alpha=ALPHA)
                    ps = mm_psum([MW(f"{w2n}_k0"), MW(f"{w2n}_k1")],
                                 [hmid[:, 0, :], hmid[:, 1, :]], (opad, N))
                    no_sb = psq.tile([14, N], F32, tag="nosb" + w1n)
                    nc.scalar.activation(no_sb[:odim, :], ps[:odim, :],
                                         A.Identity, bias=BC(b2n, h=odim))
                    nc.sync.dma_start(d_on[b][obase : obase + odim, :], no_sb[:odim, :])

                for s in range(NME):
                    sl = slice(s * ME_F, (s + 1) * ME_F)
                    ps = mm_psum([MW("out_e1_k0")], [edges_t[:, sl]], (128, ME_F))
                    oemid = pio.tile([128, ME_F], F32R, tag="pio_r")
                    nc.scalar.activation(oemid[:], ps[:], A.Prelu,
                                         bias=BC("out_e_b1"), alpha=ALPHA)
                    ps = mm_psum([MW("out_e2_k0")], [oemid[:]], (8, ME_F))
                    oe_sb = pio.tile([EDGE_D, ME_F], F32, tag="pio_o")
                    nc.scalar.activation(oe_sb[:], ps[:EDGE_D, :], A.Identity,
                                         bias=BC("out_e_b2", h=EDGE_D))
                    nc.sync.dma_start(d_oe[b][:, sl], oe_sb[:])

    import bass_rust as _br
    from concourse.hw_specs import get_activation_tables

    def _act_tables_pinned():
        tables = list(get_activation_tables(nc.m.arch).items())
        keep = {"natural_log_exp_and_others", "trig_and_small"}
        _br.insert_act_table_loads(
            nc, [(n, (f if n in keep else set())) for n, f in tables])

    nc.insert_act_table_loads = _act_tables_pinned
    nc.compile()
    return nc


_PROG_CACHE = {}


def _get_program(misc_off, misc_cols, la_off, la_cols, lb_off, lb_cols, bc_off, bc_cols, taps=False):
    key = (misc_cols, la_cols, lb_cols, bc_cols, taps)
    if key not in _PROG_CACHE:
        _PROG_CACHE[key] = _build_program(
            misc_off, misc_cols, la_off, la_cols, lb_off, lb_cols, bc_off, bc_cols, taps=taps)
    return _PROG_CACHE[key]


def kernel(nodes, edges, timestep, params, _taps=False, _trace=False):
    global LAST_RESULTS
    nodes = np.asarray(nodes, np.float32)
    edges = np.asarray(edges, np.float32)
    timestep = np.asarray(timestep)

    misc, layer_wa, laya, layer_wb, layb, bc = _pack_host(params)
    misc_blob = misc.blob()
    bc_blob = bc.blob()

    nc = _get_program(misc.off, misc_blob.shape[1], laya.off, layer_wa.shape[2],
                      layb.off, layer_wb.shape[2], bc.off, bc_blob.shape[1], taps=_taps)

    nodes_fm = np.ascontiguousarray(nodes.transpose(0, 2, 1))          # [B,20,96]
    edges_fm = np.zeros((B, 8, NN), np.float32)
    edges_fm[:, :EDGE_D, :] = edges.reshape(B, NN, EDGE_D).transpose(0, 2, 1)
    t_f32 = timestep.astype(np.float32)

    in_maps = []
    for c in range(NCORES):
        bs = slice(c * BLOC, (c + 1) * BLOC)
        in_maps.append({
            "misc_w": misc_blob,
            "layer_wa": layer_wa,
            "layer_wb": layer_wb,
            "bias_c": bc_blob,
            "nodes_fm": nodes_fm[bs],
            "edges_fm": edges_fm[bs],
            "t_f32": np.broadcast_to(t_f32[bs], (4, BLOC)).copy(),
        })

    res = bass_utils.run_bass_kernel_spmd(
        nc, in_maps, core_ids=list(range(NCORES)), trace=_trace)
    LAST_RESULTS = res

    on = np.concatenate([r["out_nodes_fm"] for r in res.results])      # [B,20,96]
    oe = np.concatenate([r["out_edges_fm"] for r in res.results])      # [B,5,9216]
    out_nodes = np.ascontiguousarray(on.transpose(0, 2, 1))            # [B,96,20]
    out_edges = np.ascontiguousarray(
        oe.transpose(0, 2, 1).reshape(B, N, N, EDGE_D))
    return out_nodes, out_edges


# revision 9
# speedup vs baseline: 1.2489x; 1.0268x over previous
# Trainium2 Bass kernel for nn_DiffusionModel_88948772700534 (gnn_message_passing).
#
# Strategy: data-parallel over batch B=16 across 8 NeuronCores (2 batches/core).
# All activations are kept feature-major on-chip ([feature_partition, token]) so
# every Linear becomes PE matmuls accumulated over K in PSUM, with bias +
# LeakyReLU (Prelu) fused into a single ScalarE activation reading PSUM.
# Matmuls run in float32r (full-rate fp32); weights are streamed per layer from
# HBM. The pairwise (i,j) tensors are processed in chunks of 4 i-rows (384
# pairs) so nothing pairwise is ever materialized in HBM; gf (the 640-dim
# concat) is never materialized at all - the f-matmul reads the edge state
# plus broadcast APs of the node state directly.
#
# sqrt and 1/x are computed as exp(0.5*ln(x)) / exp(-ln(x)) so that the whole
# kernel stays inside one ScalarE activation table set
# (natural_log_exp_and_others: exp/ln/square/identity/parametric_relu),
# avoiding ~2.7us table reloads. Softmax uses unnormalized exp (wn >= 0 is
# small) with the denominator folded in after the attention reduction.

import os
import sys
import types

for _p in ("/opt/trn_rl_repo",):
    if _p not in sys.path:
        sys.path.insert(0, _p)

# NTFF profile hook (lets BASS_TRACE=1 capture HW timing under axon).
try:
    import antenv

    if "antenv.axon_hooks" not in sys.modules:
        _hooks = types.ModuleType("antenv.axon_hooks")
        _hook_slot = [None]
        _hooks.set_axon_ntff_profile_hook = lambda h: _hook_slot.__setitem__(0, h)
        _hooks.get_axon_ntff_profile_hook = lambda: _hook_slot[0]
        sys.modules["antenv.axon_hooks"] = _hooks
        antenv.axon_hooks = _hooks
        try:
            from trn_agent_boot.trn_boot import _ntff_profile_via_ctypes

            _h = _ntff_profile_via_ctypes("/opt/axon/libaxon_pjrt.so")
            if _h is not None:
                _hooks.set_axon_ntff_profile_hook(_h)
        except Exception:
            pass
except Exception:
    pass

import numpy as np

import concourse.bass as bass
import concourse.tile as tile
from concourse import bacc, bass_utils, mybir

F32 = mybir.dt.float32
F32R = mybir.dt.float32r
BF16 = mybir.dt.bfloat16
I32 = mybir.dt.int32
A = mybir.ActivationFunctionType
OP = mybir.AluOpType
AX = mybir.AxisListType

B, N = 16, 96
NODE_D, EDGE_D = 20, 5
DN, DE, DC = 256, 128, 256
H, L, TMAX = 8, 4, 1000
EPS = 1e-5
ALPHA = 0.01
NCORES = 8
BLOC = B // NCORES          # batches per core
NN = N * N                  # 9216 pairs
CH = 5                      # i-rows per chunk (last chunk is ragged)
CF = CH * N                 # max chunk free size (480)
CHUNKS = [(i0, min(CH, N - i0)) for i0 in range(0, N, CH)]
NCHUNK = len(CHUNKS)        # 20
ME_F = 512                  # free size for 128-feature edge passes
NME = NN // ME_F            # 18

LAST_RESULTS = None         # filled by kernel() for inspection (exec_time_ns etc.)


class _Packer:
    """Packs 2D slabs [h<=128, w] into one [128, cols] blob, column-major."""

    def __init__(self):
        self.cols = 0
        self.off = {}
        self.parts = []

    def add(self, name, arr):
        arr = np.asarray(arr, np.float32)
        assert arr.ndim == 2 and arr.shape[0] <= 128
        self.off[name] = (self.cols, arr.shape[1], arr.shape[0])
        self.parts.append((self.cols, arr))
        self.cols += arr.shape[1]

    def blob(self, cols=None):
        out = np.zeros((128, cols or self.cols), np.float32)
        for c, arr in self.parts:
            out[: arr.shape[0], c : c + arr.shape[1]] = arr
        return out


def _add_lin(pk, name, w):
    """Linear weight w [O, I] -> k-slabs of w.T, named name_k{k}."""
    wT = np.asarray(w, np.float32).T  # [I, O]
    I_, O = wT.shape
    nk = (I_ + 127) // 128
    for k in range(nk):
        pk.add(f"{name}_k{k}", wT[k * 128 : (k + 1) * 128, :])


def _add_bias(pk, name, v):
    """Bias/norm vector [O] -> one column per 128-block, named name (width nm)."""
    v = np.asarray(v, np.float32).reshape(-1)
    O = v.shape[0]
    nm = (O + 127) // 128
    cols = np.zeros((128, nm), np.float32)
    for m in range(nm):
        seg = v[m * 128 : (m + 1) * 128]
        cols[: seg.shape[0], m] = seg
    pk.add(name, cols)


def _pack_host(params):
    """Build the three weight blobs + offset maps from the params pytree."""
    p = {k: params[k] for k in params}

    misc = _Packer()
    _add_lin(misc, "in_n1", p["in_n"][0])
    _add_lin(misc, "in_n2", p["in_n"][2])
    w_ie1 = np.zeros((128, 8), np.float32)
    w_ie1[:, :EDGE_D] = np.asarray(p["in_e"][0], np.float32)
    _add_lin(misc, "in_e1", w_ie1)
    _add_lin(misc, "in_e2", p["in_e"][2])
    _add_lin(misc, "in_c1", p["in_c"][0])
    _add_lin(misc, "in_c2", p["in_c"][2])
    _add_lin(misc, "out_t1", p["out_t"][0])
    w_ot2 = np.zeros((8, DN), np.float32)
    w_ot2[:6] = np.asarray(p["out_t"][2], np.float32)
    _add_lin(misc, "out_t2", w_ot2)
    _add_lin(misc, "out_p1", p["out_p"][0])
    w_op2 = np.zeros((16, DN), np.float32)
    w_op2[:14] = np.asarray(p["out_p"][2], np.float32)
    _add_lin(misc, "out_p2", w_op2)
    _add_lin(misc, "out_e1", p["out_e"][0])
    w_oe2 = np.zeros((8, DE), np.float32)
    w_oe2[:EDGE_D] = np.asarray(p["out_e"][2], np.float32)
    _add_lin(misc, "out_e2", w_oe2)
    # E8: [256, 8] block-diag ones (sum w^2 within each head) -> 2 k-slabs
    e8 = np.zeros((256, 8), np.float32)
    for hd in range(256):
        e8[hd, hd // 32] = 1.0
    _add_lin(misc, "E8", e8.T)          # treat as w [8, 256]
    # Eexp: lhsT [8, 256] (broadcast head value to its 32 dims)
    _add_lin(misc, "Eexp", e8)          # w [256, 8] -> wT [8, 256] single slab
    misc.add("ones1", np.full((4, 128), 0.25, np.float32))

    laya = layb = None
    a_blobs, b_blobs = [], []
    for li in range(L):
        lp = p["layers"][li]
        mha = lp["mha"]
        pa = _Packer()
        _add_lin(pa, "f", mha["f"][0])
        _add_lin(pa, "e", mha["e"][0])
        _add_lin(pa, "v1", mha["v"][0])
        _add_lin(pa, "v2", mha["v"][2])
        _add_lin(pa, "w1", mha["w"][0])
        _add_lin(pa, "w2", mha["w"][2])
        _add_lin(pa, "m", mha["m"][0])
        _add_lin(pa, "a", mha["a"][0])
        pb2 = _Packer()
        _add_lin(pb2, "o", mha["o"][0])
        _add_lin(pb2, "mn1", lp["mn"][0])
        _add_lin(pb2, "mn2", lp["mn"][2])
        _add_lin(pb2, "me1", lp["me"][0])
        _add_lin(pb2, "me2", lp["me"][2])
        if laya is None:
            laya, layb = pa, pb2
        a_blobs.append(pa.blob())
        b_blobs.append(pb2.blob())
    layer_wa = np.stack(a_blobs)
    layer_wb = np.stack(b_blobs)

    bc = _Packer()
    # time-embedding constants: s/(2pi) and phase (0 or 0.25), packed per m-col
    scales = np.exp(np.arange(0, DC, 2, dtype=np.float64) * (-np.log(10000.0) / DC))
    sp = np.repeat(scales, 2) / (2.0 * np.pi)   # [256]
    ph = np.tile([0.0, 0.25], DC // 2)          # [256]
    _add_bias(bc, "eps", np.full(128, EPS))
    _add_bias(bc, "s2pi", sp)
    _add_bias(bc, "phase", ph)
    _add_bias(bc, "in_n_b1", p["in_n"][1])
    _add_bias(bc, "in_n_b2", p["in_n"][3])
    _add_bias(bc, "in_e_b1", p["in_e"][1])
    _add_bias(bc, "in_e_b2", p["in_e"][3])
    _add_bias(bc, "in_c_b1", p["in_c"][1])
    _add_bias(bc, "in_c_b2", p["in_c"][3])
    _add_bias(bc, "out_t_b1", p["out_t"][1])
    _add_bias(bc, "out_t_b2", p["out_t"][3])
    _add_bias(bc, "out_p_b1", p["out_p"][1])
    _add_bias(bc, "out_p_b2", p["out_p"][3])
    _add_bias(bc, "out_e_b1", p["out_e"][1])
    _add_bias(bc, "out_e_b2", p["out_e"][3])
    for li in range(L):
        lp = p["layers"][li]
        mha = lp["mha"]
        _add_bias(bc, f"l{li}_bf", mha["f"][1])
        _add_bias(bc, f"l{li}_be", mha["e"][1])
        _add_bias(bc, f"l{li}_bv1", mha["v"][1])
        _add_bias(bc, f"l{li}_bv2", mha["v"][3])
        _add_bias(bc, f"l{li}_bw1", mha["w"][1])
        _add_bias(bc, f"l{li}_bw2", mha["w"][3])
        _add_bias(bc, f"l{li}_bo", mha["o"][1])
        _add_bias(bc, f"l{li}_bm", mha["m"][1])
        _add_bias(bc, f"l{li}_ba", mha["a"][1])
        _add_bias(bc, f"l{li}_bmn1", lp["mn"][1])
        _add_bias(bc, f"l{li}_bmn2", lp["mn"][3])
        _add_bias(bc, f"l{li}_bme1", lp["me"][1])
        _add_bias(bc, f"l{li}_bme2", lp["me"][3])
        for nn_, wdt in (("n1", DN), ("n2", DE), ("n3", DN), ("n4", DE)):
            g, b_ = lp[nn_]
            _add_bias(bc, f"l{li}_{nn_}g", g)
            _add_bias(bc, f"l{li}_{nn_}b", b_)

    return misc, layer_wa, laya, layer_wb, layb, bc


def _build_program(misc_off, misc_cols, la_off, la_cols, lb_off, lb_cols, bc_off, bc_cols, taps=False):
    nc = bacc.Bacc("TRN2", debug=False)

    d_misc = nc.dram_tensor("misc_w", [128, misc_cols], F32R, kind="ExternalInput").ap()
    d_laya = nc.dram_tensor("layer_wa", [L, 128, la_cols], F32R, kind="ExternalInput").ap()
    d_layb = nc.dram_tensor("layer_wb", [L, 128, lb_cols], F32R, kind="ExternalInput").ap()
    d_bc = nc.dram_tensor("bias_c", [128, bc_cols], F32, kind="ExternalInput").ap()
    d_nodes = nc.dram_tensor("nodes_fm", [BLOC, NODE_D, N], F32R, kind="ExternalInput").ap()
    d_edges = nc.dram_tensor("edges_fm", [BLOC, 8, NN], F32R, kind="ExternalInput").ap()
    d_t = nc.dram_tensor("t_f32", [4, BLOC], F32R, kind="ExternalInput").ap()
    d_on = nc.dram_tensor("out_nodes_fm", [BLOC, NODE_D, N], F32, kind="ExternalOutput").ap()
    d_oe = nc.dram_tensor("out_edges_fm", [BLOC, EDGE_D, NN], F32, kind="ExternalOutput").ap()

    tapd = {}

    def tap(name, shape):
        if taps:
            tapd[name] = nc.dram_tensor(name, shape, F32, kind="ExternalOutput").ap()
        return tapd.get(name)

    with tile.TileContext(nc) as tc:
        with (
            tc.tile_pool(name="pw_misc", bufs=1) as pw_misc,
            tc.tile_pool(name="pw_laya", bufs=2) as pw_laya,
            tc.tile_pool(name="pw_layb", bufs=1) as pw_layb,
            tc.tile_pool(name="pb", bufs=1) as pbp,
            tc.tile_pool(name="pstate", bufs=1) as pstate,
            tc.tile_pool(name="pchunk", bufs=2) as pch,
            tc.tile_pool(name="pchunk1", bufs=1) as pch1,
            tc.tile_pool(name="psmall", bufs=2) as psm,
            tc.tile_pool(name="pseq", bufs=1) as psq,
            tc.tile_pool(name="pio", bufs=2) as pio,
            tc.tile_pool(name="pp", bufs=8, space="PSUM") as pp,
        ):
            mw = pw_misc.tile([128, misc_cols], F32R, tag="misc")
            nc.sync.dma_start(mw[:], d_misc[:])
            bcw = pbp.tile([128, bc_cols], F32, tag="bias")
            nc.sync.dma_start(bcw[:], d_bc[:])

            def MW(name, m=0, mw_width=128):
                off, w, h = misc_off[name]
                return mw[:h, off + m * mw_width : off + min((m + 1) * mw_width, w)]

            def BC(name, m=0, h=128):
                off, w, _ = bc_off[name]
                return bcw[:h, off + m : off + m + 1]

            def mm_psum(lhs_list, rhs_list, out_shape=(128, CF)):
                """Accumulating matmul chain into a fresh psum tile."""
                ps = pp.tile(list(out_shape), F32, tag="mm")
                nk = len(lhs_list)
                for k in range(nk):
                    nc.tensor.matmul(
                        ps[:], lhs_list[k], rhs_list[k],
                        start=(k == 0), stop=(k == nk - 1),
                    )
                return ps

            # ============ per-core program: loop over local batches ============
            for b in range(BLOC):
                # ---- persistent per-batch state tiles (reused across batches) --
                edges_t = pstate.tile([128, NN], F32R, tag="edges")
                ae_t = pstate.tile([128, NN], F32, tag="ae")
                nodes_t = psq.tile([128, 2, N], F32R, tag="nodes")

                # ---------------- time embedding + cond -----------------------
                t_sb = psm.tile([4, BLOC], F32R, tag="tsb")
                nc.sync.dma_start(t_sb[:], d_t[:])
                ps_t = pp.tile([128, BLOC], F32, tag="mm")
                nc.tensor.matmul(ps_t[:], MW("ones1"), t_sb[:], start=True, stop=True)
                emb = psq.tile([128, 2, BLOC], F32R, tag="emb")
                for m in range(2):
                    y = psm.tile([128, BLOC], F32, tag="y")
                    nc.scalar.activation(y[:], ps_t[:], A.Identity,
                                         bias=BC("phase", m), scale=BC("s2pi", m))
                    ki = psm.tile([128, BLOC], I32, tag="ki")
                    nc.vector.tensor_copy(ki[:], y[:])
                    kf = psm.tile([128, BLOC], F32, tag="kf")
                    nc.vector.tensor_copy(kf[:], ki[:])
                    fr = psm.tile([128, BLOC], F32, tag="fr")
                    nc.vector.tensor_sub(fr[:], y[:], kf[:])
                    nc.scalar.activation(emb[:, m, :], fr[:], A.Sin, scale=float(2 * np.pi))
                cmid = psq.tile([128, 2, BLOC], F32R, tag="cmid")
                for m in range(2):
                    ps = mm_psum([MW("in_c1_k0", m), MW("in_c1_k1", m)],
                                 [emb[:, 0, :], emb[:, 1, :]], (128, BLOC))
                    nc.scalar.activation(cmid[:, m, :], ps[:], A.Prelu,
                                         bias=BC("in_c_b1", m), alpha=ALPHA)
                cond = psq.tile([128, 2, BLOC], F32R, tag="cond")
                for m in range(2):
                    ps = mm_psum([MW("in_c2_k0", m), MW("in_c2_k1", m)],
                                 [cmid[:, 0, :], cmid[:, 1, :]], (128, BLOC))
                    nc.scalar.activation(cond[:, m, :], ps[:], A.Prelu,
                                         bias=BC("in_c_b2", m), alpha=ALPHA)
                if b == 0:
                    t_ = tap("tap_cond", [128, 2, BLOC])
                    if t_ is not None:
                        nc.sync.dma_start(t_[:], cond[:])

                # ---------------- input MLPs ---------------------------------
                nraw = psq.tile([NODE_D, N], F32R, tag="nraw")
                nc.sync.dma_start(nraw[:], d_nodes[b])
                nmid = psq.tile([128, 2, N], F32R, tag="nmid")
                for m in range(2):
                    ps = mm_psum([MW("in_n1_k0", m)], [nraw[:]], (128, N))
                    nc.scalar.activation(nmid[:, m, :], ps[:], A.Prelu,
                                         bias=BC("in_n_b1", m), alpha=ALPHA)
                for m in range(2):
                    ps = mm_psum([MW("in_n2_k0", m), MW("in_n2_k1", m)],
                                 [nmid[:, 0, :], nmid[:, 1, :]], (128, N))
                    nc.scalar.activation(nodes_t[:, m, :], ps[:], A.Prelu,
                                         bias=BC("in_n_b2", m), alpha=ALPHA)

                for s in range(NME):
                    sl = slice(s * ME_F, (s + 1) * ME_F)
                    eraw = pio.tile([8, ME_F], F32R, tag="pio_e")
                    nc.sync.dma_start(eraw[:], d_edges[b][:, sl])
                    ps = mm_psum([MW("in_e1_k0")], [eraw[:]], (128, ME_F))
                    emid = pio.tile([128, ME_F], F32R, tag="pio_r")
                    nc.scalar.activation(emid[:], ps[:], A.Prelu,
                                         bias=BC("in_e_b1"), alpha=ALPHA)
                    ps = mm_psum([MW("in_e2_k0")], [emid[:]], (128, ME_F))
                    nc.scalar.activation(edges_t[:, sl], ps[:], A.Prelu,
                                         bias=BC("in_e_b2"), alpha=ALPHA)

                if b == 0:
                    t_ = tap("tap_nodes_in", [128, 2, N])
                    if t_ is not None:
                        nc.sync.dma_start(t_[:], nodes_t[:])
                    t_ = tap("tap_edges_in", [128, 512])
                    if t_ is not None:
                        nc.sync.dma_start(t_[:], edges_t[:, 0:512])

                # ================= layers =====================================
                for li in range(L):
                    lwa = pw_laya.tile([128, la_cols], F32R, tag="laywa")
                    nc.sync.dma_start(lwa[:], d_laya[li])
                    lwb = pw_layb.tile([128, lb_cols], F32R, tag="laywb")
                    nc.sync.dma_start(lwb[:], d_layb[li])

                    def LW(name, m=0, _la=lwa, _lb=lwb):
                        if name.split("_k")[0] in ("o", "mn1", "mn2", "me1", "me2"):
                            off, w, h = lb_off[name]
                            return _lb[:h, off + m * 128 : off + min((m + 1) * 128, w)]
                        off, w, h = la_off[name]
                        return _la[:h, off + m * 128 : off + min((m + 1) * 128, w)]

                    pfx = f"l{li}_"

                    # ---- FiLM params: mul/add = cond @ Wm/Wa + b -------------
                    mul_sb = psm.tile([128, 2, BLOC], F32, tag="mul")
                    filmb = psm.tile([128, 2, BLOC], F32, tag="filmb")
                    add_sb = psm.tile([128, 2, BLOC], F32, tag="adds")
                    for m in range(2):
                        ps = mm_psum([LW("m_k0", m), LW("m_k1", m)],
                                     [cond[:, 0, :], cond[:, 1, :]], (128, BLOC))
                        nc.scalar.activation(mul_sb[:, m, :], ps[:], A.Identity,
                                             bias=BC(pfx + "bm", m))
                        ps = mm_psum([LW("a_k0", m), LW("a_k1", m)],
                                     [cond[:, 0, :], cond[:, 1, :]], (128, BLOC))
                        nc.scalar.activation(add_sb[:, m, :], ps[:], A.Identity,
                                             bias=BC(pfx + "ba", m))
                        # filmb = bf * mul + add  (per batch column)
                        for bb in range(BLOC):
                            nc.vector.tensor_scalar(
                                filmb[:, m, bb : bb + 1], BC(pfx + "bf", m),
                                mul_sb[:, m, bb : bb + 1], add_sb[:, m, bb : bb + 1],
                                op0=OP.mult, op1=OP.add)

                    # ---- chunk loop over i-rows ------------------------------
                    st1 = psq.tile([128, NCHUNK, 6], F32, tag="st1")
                    denom = psq.tile([8, N], F32, tag="denom")
                    out_raw = psq.tile([128, 2, N], F32, tag="oraw")
                    for c, (i0, ch) in enumerate(CHUNKS):
                        cf = ch * N
                        csl = slice(i0 * N, i0 * N + cf)
                        # f = Prelu((gf @ Wf.T) * mul + (bf*mul+add))
                        f_sb = pch.tile([128, 2, cf], F32R, tag="f_sb")
                        for m in range(2):
                            lhs = [LW(f"f_k{k}", m) for k in range(5)]
                            rhs = [
                                edges_t[:, csl],
                                nodes_t[:, 0, :].unsqueeze(1).broadcast_to([128, ch, N]),
                                nodes_t[:, 1, :].unsqueeze(1).broadcast_to([128, ch, N]),
                                nodes_t[:, 0, i0 : i0 + ch].unsqueeze(2).broadcast_to([128, ch, N]),
                                nodes_t[:, 1, i0 : i0 + ch].unsqueeze(2).broadcast_to([128, ch, N]),
                            ]
                            ps = mm_psum(lhs, rhs, (128, cf))
                            nc.scalar.activation(f_sb[:, m, :], ps[:], A.Prelu,
                                                 bias=filmb[:, m, b : b + 1],
                                                 scale=mul_sb[:, m, b : b + 1], alpha=ALPHA)
                        if taps and b == 0 and li == 0 and c == 0:
                            t_ = tap("tap_f0", [128, 2, cf])
                            nc.vector.tensor_copy(tapdbg_f := pch.tile([128, 2, cf], F32, tag="tapf"), f_sb[:])
                            nc.sync.dma_start(t_[:], tapdbg_f[:])
                        # ae = Prelu(f @ We.T + be); x1 = edges + ae; stats
                        ps = mm_psum([LW("e_k0"), LW("e_k1")],
                                     [f_sb[:, 0, :], f_sb[:, 1, :]], (128, cf))
                        nc.scalar.activation(ae_t[:, csl], ps[:], A.Prelu,
                                             bias=BC(pfx + "be"), alpha=ALPHA)
                        nc.vector.tensor_add(ae_t[:, csl], edges_t[:, csl], ae_t[:, csl])
                        nc.vector.bn_stats(st1[:, c, :], ae_t[:, csl])
                        # v mlp
                        vmid = pch.tile([128, 2, cf], F32R, tag="vmid")
                        for m in range(2):
                            ps = mm_psum([LW("v1_k0", m), LW("v1_k1", m)],
                                         [f_sb[:, 0, :], f_sb[:, 1, :]], (128, cf))
                            nc.scalar.activation(vmid[:, m, :], ps[:], A.Prelu,
                                                 bias=BC(pfx + "bv1", m), alpha=ALPHA)
                        v_sb = pch.tile([128, 2, cf], F32, tag="v_sb")
                        for m in range(2):
                            ps = mm_psum([LW("v2_k0", m), LW("v2_k1", m)],
                                         [vmid[:, 0, :], vmid[:, 1, :]], (128, cf))
                            nc.vector.tensor_scalar(v_sb[:, m, :], ps[:],
                                                    BC(pfx + "bv2", m), 0.0,
                                                    op0=OP.add, op1=OP.add)
                        # w mlp -> w^2
                        wmid = pch.tile([128, 2, cf], F32R, tag="wmid")
                        for m in range(2):
                            ps = mm_psum([LW("w1_k0", m), LW("w1_k1", m)],
                                         [f_sb[:, 0, :], f_sb[:, 1, :]], (128, cf))
                            nc.scalar.activation(wmid[:, m, :], ps[:], A.Prelu,
                                                 bias=BC(pfx + "bw1", m), alpha=ALPHA)
                        w2sb = pch.tile([128, 2, cf], F32R, tag="w2sb")
                        for m in range(2):
                            ps = mm_psum([LW("w2_k0", m), LW("w2_k1", m)],
                                         [wmid[:, 0, :], wmid[:, 1, :]], (128, cf))
                            nc.scalar.activation(w2sb[:, m, :], ps[:], A.Square,
                                                 bias=BC(pfx + "bw2", m))
                        # wn2 -> exp(sqrt(q/256))
                        ps_wn = mm_psum([MW("E8_k0"), MW("E8_k1")],
                                        [w2sb[:, 0, :], w2sb[:, 1, :]], (8, cf))
                        expc = pch.tile([8, cf], F32R, tag="expc")
                        nc.scalar.activation(expc[:], ps_wn[:], A.Ln, scale=1.0 / DN)
                        nc.scalar.activation(expc[:], expc[:], A.Exp, scale=0.5)
                        nc.scalar.activation(expc[:], expc[:], A.Exp)
                        nc.vector.tensor_reduce(
                            denom[:, i0 : i0 + ch],
                            expc[:].rearrange("p (i j) -> p i j", i=ch),
                            axis=AX.X, op=OP.add)
                        # attention accumulate: out_raw += sum_j exp * v
                        for m in range(2):
                            ps_x = pp.tile([128, cf], F32, tag="mm")
                            nc.tensor.matmul(ps_x[:], MW("Eexp_k0", m), expc[:],
                                             start=True, stop=True)
                            pv = pch1.tile([128, cf], F32, tag="pv")
                            nc.vector.tensor_mul(pv[:], ps_x[:], v_sb[:, m, :])
                            nc.vector.tensor_reduce(
                                out_raw[:, m, i0 : i0 + ch],
                                pv[:].rearrange("p (i j) -> p i j", i=ch),
                                axis=AX.X, op=OP.add)

                    # ---- attention finalize + node path ----------------------
                    lnden = psm.tile([8, N], F32, tag="lnden")
                    nc.scalar.activation(lnden[:], denom[:], A.Ln)
                    rden = psm.tile([8, N], F32R, tag="rden")
                    nc.scalar.activation(rden[:], lnden[:], A.Exp, scale=-1.0)
                    att = psq.tile([128, 2, N], F32R, tag="att")
                    for m in range(2):
                        ps_r = pp.tile([128, N], F32, tag="mm")
                        nc.tensor.matmul(ps_r[:], MW("Eexp_k0", m), rden[:],
                                         start=True, stop=True)
                        nc.vector.tensor_mul(att[:, m, :], out_raw[:, m, :], ps_r[:])
                    an = psq.tile([128, 2, N], F32, tag="an")
                    for m in range(2):
                        ps = mm_psum([LW("o_k0", m), LW("o_k1", m)],
                                     [att[:, 0, :], att[:, 1, :]], (128, N))
                        nc.scalar.activation(an[:, m, :], ps[:], A.Prelu,
                                             bias=BC(pfx + "bo", m), alpha=ALPHA)
                    if taps and b == 0 and li == 0:
                        t_ = tap("tap_att", [128, 2, N])
                        nc.sync.dma_start(t_[:], an[:])
                        t_ = tap("tap_denom", [8, N])
                        nc.sync.dma_start(t_[:], denom[:])

                    def node_norm(x_t, gname, bname):
                        """Instance-norm x_t [128,2,N] over N per feature -> nodes_t."""
                        for m in range(2):
                            stn = psm.tile([128, 6], F32, tag="stn")
                            nc.vector.bn_stats(stn[:], x_t[:, m, :])
                            mv = psm.tile([128, 2], F32, tag="mvn")
                            nc.vector.bn_aggr(mv[:], stn[:])
                            lnv = psm.tile([128, 1], F32, tag="lnvn")
                            nc.scalar.activation(lnv[:], mv[:, 1:2], A.Ln, bias=BC("eps"))
                            rstd = psm.tile([128, 1], F32, tag="rstdn")
                            nc.scalar.activation(rstd[:], lnv[:], A.Exp, scale=-0.5)
                            s_ = psm.tile([128, 1], F32, tag="sn")
                            nc.vector.tensor_mul(s_[:], rstd[:], BC(gname, m))
                            ms = psm.tile([128, 1], F32, tag="msn")
                            nc.vector.tensor_mul(ms[:], mv[:, 0:1], s_[:])
                            tt_ = psm.tile([128, 1], F32, tag="tn")
                            nc.vector.tensor_sub(tt_[:], BC(bname, m), ms[:])
                            nc.scalar.activation(nodes_t[:, m, :], x_t[:, m, :],
                                                 A.Identity, bias=tt_[:, 0:1],
                                                 scale=s_[:, 0:1])

                    x1n = psq.tile([128, 2, N], F32, tag="x1n")
                    nc.vector.tensor_add(x1n[:], nodes_t[:], an[:])
                    node_norm(x1n, pfx + "n1g", pfx + "n1b")

                    nmid2 = psq.tile([128, 2, N], F32R, tag="nmid2")
                    for m in range(2):
                        ps = mm_psum([LW("mn1_k0", m), LW("mn1_k1", m)],
                                     [nodes_t[:, 0, :], nodes_t[:, 1, :]], (128, N))
                        nc.scalar.activation(nmid2[:, m, :], ps[:], A.Prelu,
                                             bias=BC(pfx + "bmn1", m), alpha=ALPHA)
                    x2n = psq.tile([128, 2, N], F32, tag="x2n")
                    for m in range(2):
                        ps = mm_psum([LW("mn2_k0", m), LW("mn2_k1", m)],
                                     [nmid2[:, 0, :], nmid2[:, 1, :]], (128, N))
                        tmp = psm.tile([128, N], F32, tag="tmpn")
                        nc.scalar.activation(tmp[:], ps[:], A.Prelu,
                                             bias=BC(pfx + "bmn2", m), alpha=ALPHA)
                        nc.vector.tensor_add(x2n[:, m, :], nodes_t[:, m, :], tmp[:])
                    node_norm(x2n, pfx + "n3g", pfx + "n3b")

                    # ---- edge norm n2 (x1 already in ae_t, stats in st1) -----
                    def edge_norm(stats_tile, gname, bname):
                        mv = psm.tile([128, 2], F32, tag="mve")
                        nc.vector.bn_aggr(mv[:], stats_tile[:])
                        lnv = psm.tile([128, 1], F32, tag="lnve")
                        nc.scalar.activation(lnv[:], mv[:, 1:2], A.Ln, bias=BC("eps"))
                        rstd = psm.tile([128, 1], F32, tag="rstde")
                        nc.scalar.activation(rstd[:], lnv[:], A.Exp, scale=-0.5)
                        s_ = psm.tile([128, 1], F32, tag="se")
                        nc.vector.tensor_mul(s_[:], rstd[:], BC(gname))
                        ms = psm.tile([128, 1], F32, tag="mse")
                        nc.vector.tensor_mul(ms[:], mv[:, 0:1], s_[:])
                        tt_ = psm.tile([128, 1], F32, tag="te")
                        nc.vector.tensor_sub(tt_[:], BC(bname), ms[:])
                        nc.scalar.activation(edges_t[:], ae_t[:], A.Identity,
                                             bias=tt_[:, 0:1], scale=s_[:, 0:1])

                    edge_norm(st1, pfx + "n2g", pfx + "n2b")

                    # ---- me mlp + residual + n4 ------------------------------
                    st2 = psq.tile([128, NME, 6], F32, tag="st2")
                    for s in range(NME):
                        sl = slice(s * ME_F, (s + 1) * ME_F)
                        ps = mm_psum([LW("me1_k0")], [edges_t[:, sl]], (128, ME_F))
                        memid = pio.tile([128, ME_F], F32R, tag="pio_r")
                        nc.scalar.activation(memid[:], ps[:], A.Prelu,
                                             bias=BC(pfx + "bme1"), alpha=ALPHA)
                        ps = mm_psum([LW("me2_k0")], [memid[:]], (128, ME_F))
                        meout = pio.tile([128, ME_F], F32, tag="pio_f")
                        nc.scalar.activation(meout[:], ps[:], A.Prelu,
                                             bias=BC(pfx + "bme2"), alpha=ALPHA)
                        nc.vector.tensor_add(ae_t[:, sl], edges_t[:, sl], meout[:])
                        nc.vector.bn_stats(st2[:, s, :], ae_t[:, sl])
                    edge_norm(st2, pfx + "n4g", pfx + "n4b")

                    if taps and b == 0 and li == 0:
                        t_ = tap("tap_nodes_l0", [128, 2, N])
                        nc.sync.dma_start(t_[:], nodes_t[:])
                        t_ = tap("tap_edges_l0", [128, 512])
                        nc.sync.dma_start(t_[:], edges_t[:, 0:512])

                # ================= output heads ===============================
                for w1n, w2n, b1n, b2n, odim, opad, obase in (
                    ("out_t1", "out_t2", "out_t_b1", "out_t_b2", 6, 8, 0),
                    ("out_p1", "out_p2", "out_p_b1", "out_p_b2", 14, 16, 6),
                ):
                    hmid = psq.tile([128, 2, N], F32R, tag="hmid")
                    for m in range(2):
                        ps = mm_psum([MW(f"{w1n}_k0", m), MW(f"{w1n}_k1", m)],
                                     [nodes_t[:, 0, :], nodes_t[:, 1, :]], (128, N))
                        nc.scalar.activation(hmid[:, m, :], ps[:], A.Prelu,
                                             bias=BC(b1n, m), alpha=ALPHA)
                    ps = mm_psum([MW(f"{w2n}_k0"), MW(f"{w2n}_k1")],
                                 [hmid[:, 0, :], hmid[:, 1, :]], (opad, N))
                    no_sb = psq.tile([14, N], F32, tag="nosb" + w1n)
                    nc.scalar.activation(no_sb[:odim, :], ps[:odim, :],
                                         A.Identity, bias=BC(b2n, h=odim))
                    nc.sync.dma_start(d_on[b][obase : obase + odim, :], no_sb[:odim, :])

                for s in range(NME):
                    sl = slice(s * ME_F, (s + 1) * ME_F)
                    ps = mm_psum([MW("out_e1_k0")], [edges_t[:, sl]], (128, ME_F))
                    oemid = pio.tile([128, ME_F], F32R, tag="pio_r")
                    nc.scalar.activation(oemid[:], ps[:], A.Prelu,
                                         bias=BC("out_e_b1"), alpha=ALPHA)
                    ps = mm_psum([MW("out_e2_k0")], [oemid[:]], (8, ME_F))
                    oe_sb = pio.tile([EDGE_D, ME_F], F32, tag="pio_o")
                    nc.scalar.activation(oe_sb[:], ps[:EDGE_D, :], A.Identity,
                                         bias=BC("out_e_b2", h=EDGE_D))
                    nc.sync.dma_start(d_oe[b][:, sl], oe_sb[:])

    import bass_rust as _br
    from concourse.hw_specs import get_activation_tables

    def _act_tables_pinned():
        tables = list(get_activation_tables(nc.m.arch).items())
        keep = {"natural_log_exp_and_others", "trig_and_small"}
        _br.insert_act_table_loads(
            nc, [(n, (f if n in keep else set())) for n, f in tables])

    nc.insert_act_table_loads = _act_tables_pinned
    nc.compile()
    return nc


_PROG_CACHE = {}


def _get_program(misc_off, misc_cols, la_off, la_cols, lb_off, lb_cols, bc_off, bc_cols, taps=False):
    key = (misc_cols, la_cols, lb_cols, bc_cols, taps)
    if key not in _PROG_CACHE:
        _PROG_CACHE[key] = _build_program(
            misc_off, misc_cols, la_off, la_cols, lb_off, lb_cols, bc_off, bc_cols, taps=taps)
    return _PROG_CACHE[key]


def kernel(nodes, edges, timestep, params, _taps=False, _trace=False):
    global LAST_RESULTS
    nodes = np.asarray(nodes, np.float32)
    edges = np.asarray(edges, np.float32)
    timestep = np.asarray(timestep)

    misc, layer_wa, laya, layer_wb, layb, bc = _pack_host(params)
    misc_blob = misc.blob()
    bc_blob = bc.blob()

    nc = _get_program(misc.off, misc_blob.shape[1], laya.off, layer_wa.shape[2],
                      layb.off, layer_wb.shape[2], bc.off, bc_blob.shape[1], taps=_taps)

    nodes_fm = np.ascontiguousarray(nodes.transpose(0, 2, 1))          # [B,20,96]
    edges_fm = np.zeros((B, 8, NN), np.float32)
    edges_fm[:, :EDGE_D, :] = edges.reshape(B, NN, EDGE_D).transpose(0, 2, 1)
    t_f32 = timestep.astype(np.float32)

    in_maps = []
    for c in range(NCORES):
        bs = slice(c * BLOC, (c + 1) * BLOC)
        in_maps.append({
            "misc_w": misc_blob,
            "layer_wa": layer_wa,
            "layer_wb": layer_wb,
            "bias_c": bc_blob,
            "nodes_fm": nodes_fm[bs],
            "edges_fm": edges_fm[bs],
            "t_f32": np.broadcast_to(t_f32[bs], (4, BLOC)).copy(),
        })

    res = bass_utils.run_bass_kernel_spmd(
        nc, in_maps, core_ids=list(range(NCORES)), trace=_trace)
    LAST_RESULTS = res

    on = np.concatenate([r["out_nodes_fm"] for r in res.results])      # [B,20,96]
    oe = np.concatenate([r["out_edges_fm"] for r in res.results])      # [B,5,9216]
    out_nodes = np.ascontiguousarray(on.transpose(0, 2, 1))            # [B,96,20]
    out_edges = np.ascontiguousarray(
        oe.transpose(0, 2, 1).reshape(B, N, N, EDGE_D))
    return out_nodes, out_edges


# revision 10
# speedup vs baseline: 1.2834x; 1.0276x over previous
# Trainium2 Bass kernel for nn_DiffusionModel_88948772700534 (gnn_message_passing).
#
# Strategy: data-parallel over batch B=16 across 8 NeuronCores (2 batches/core).
# All activations are kept feature-major on-chip ([feature_partition, token]) so
# every Linear becomes PE matmuls accumulated over K in PSUM, with bias +
# LeakyReLU (Prelu) fused into a single ScalarE activation reading PSUM.
# Matmuls run in float32r (full-rate fp32); weights are streamed per layer from
# HBM. The pairwise (i,j) tensors are processed in chunks of 4 i-rows (384
# pairs) so nothing pairwise is ever materialized in HBM; gf (the 640-dim
# concat) is never materialized at all - the f-matmul reads the edge state
# plus broadcast APs of the node state directly.
#
# sqrt and 1/x are computed as exp(0.5*ln(x)) / exp(-ln(x)) so that the whole
# kernel stays inside one ScalarE activation table set
# (natural_log_exp_and_others: exp/ln/square/identity/parametric_relu),
# avoiding ~2.7us table reloads. Softmax uses unnormalized exp (wn >= 0 is
# small) with the denominator folded in after the attention reduction.

import os
import sys
import types

for _p in ("/opt/trn_rl_repo",):
    if _p not in sys.path:
        sys.path.insert(0, _p)

# NTFF profile hook (lets BASS_TRACE=1 capture HW timing under axon).
try:
    import antenv

    if "antenv.axon_hooks" not in sys.modules:
        _hooks = types.ModuleType("antenv.axon_hooks")
        _hook_slot = [None]
        _hooks.set_axon_ntff_profile_hook = lambda h: _hook_slot.__setitem__(0, h)
        _hooks.get_axon_ntff_profile_hook = lambda: _hook_slot[0]
        sys.modules["antenv.axon_hooks"] = _hooks
        antenv.axon_hooks = _hooks
        try:
            from trn_agent_boot.trn_boot import _ntff_profile_via_ctypes

            _h = _ntff_profile_via_ctypes("/opt/axon/libaxon_pjrt.so")
            if _h is not None:
                _hooks.set_axon_ntff_profile_hook(_h)
        except Exception:
            pass
except Exception:
    pass

import numpy as np

import concourse.bass as bass
import concourse.tile as tile
from concourse import bacc, bass_utils, mybir

F32 = mybir.dt.float32
F32R = mybir.dt.float32r
BF16 = mybir.dt.bfloat16
I32 = mybir.dt.int32
A = mybir.ActivationFunctionType
OP = mybir.AluOpType
AX = mybir.AxisListType

B, N = 16, 96
NODE_D, EDGE_D = 20, 5
DN, DE, DC = 256, 128, 256
H, L, TMAX = 8, 4, 1000
EPS = 1e-5
ALPHA = 0.01
NCORES = 8
BLOC = B // NCORES          # batches per core
NN = N * N                  # 9216 pairs
CH = 5                      # i-rows per chunk (last chunk is ragged)
CF = CH * N                 # max chunk free size (480)
CHUNKS = [(i0, min(CH, N - i0)) for i0 in range(0, N, CH)]
NCHUNK = len(CHUNKS)        # 20
ME_F = 512                  # free size for 128-feature edge passes
NME = NN // ME_F            # 18

LAST_RESULTS = None         # filled by kernel() for inspection (exec_time_ns etc.)


class _Packer:
    """Packs 2D slabs [h<=128, w] into one [128, cols] blob, column-major."""

    def __init__(self):
        self.cols = 0
        self.off = {}
        self.parts = []

    def add(self, name, arr):
        arr = np.asarray(arr, np.float32)
        assert arr.ndim == 2 and arr.shape[0] <= 128
        self.off[name] = (self.cols, arr.shape[1], arr.shape[0])
        self.parts.append((self.cols, arr))
        self.cols += arr.shape[1]

    def blob(self, cols=None):
        out = np.zeros((128, cols or self.cols), np.float32)
        for c, arr in self.parts:
            out[: arr.shape[0], c : c + arr.shape[1]] = arr
        return out


def _add_lin(pk, name, w):
    """Linear weight w [O, I] -> k-slabs of w.T, named name_k{k}."""
    wT = np.asarray(w, np.float32).T  # [I, O]
    I_, O = wT.shape
    nk = (I_ + 127) // 128
    for k in range(nk):
        pk.add(f"{name}_k{k}", wT[k * 128 : (k + 1) * 128, :])


def _add_bias(pk, name, v):
    """Bias/norm vector [O] -> one column per 128-block, named name (width nm)."""
    v = np.asarray(v, np.float32).reshape(-1)
    O = v.shape[0]
    nm = (O + 127) // 128
    cols = np.zeros((128, nm), np.float32)
    for m in range(nm):
        seg = v[m * 128 : (m + 1) * 128]
        cols[: seg.shape[0], m] = seg
    pk.add(name, cols)


def _pack_host(params):
    """Build the three weight blobs + offset maps from the params pytree."""
    p = {k: params[k] for k in params}

    misc = _Packer()
    _add_lin(misc, "in_n1", p["in_n"][0])
    _add_lin(misc, "in_n2", p["in_n"][2])
    w_ie1 = np.zeros((128, 8), np.float32)
    w_ie1[:, :EDGE_D] = np.asarray(p["in_e"][0], np.float32)
    _add_lin(misc, "in_e1", w_ie1)
    _add_lin(misc, "in_e2", p["in_e"][2])
    _add_lin(misc, "in_c1", p["in_c"][0])
    _add_lin(misc, "in_c2", p["in_c"][2])
    _add_lin(misc, "out_t1", p["out_t"][0])
    w_ot2 = np.zeros((8, DN), np.float32)
    w_ot2[:6] = np.asarray(p["out_t"][2], np.float32)
    _add_lin(misc, "out_t2", w_ot2)
    _add_lin(misc, "out_p1", p["out_p"][0])
    w_op2 = np.zeros((16, DN), np.float32)
    w_op2[:14] = np.asarray(p["out_p"][2], np.float32)
    _add_lin(misc, "out_p2", w_op2)
    _add_lin(misc, "out_e1", p["out_e"][0])
    w_oe2 = np.zeros((8, DE), np.float32)
    w_oe2[:EDGE_D] = np.asarray(p["out_e"][2], np.float32)
    _add_lin(misc, "out_e2", w_oe2)
    # E8: [256, 8] block-diag ones (sum w^2 within each head) -> 2 k-slabs
    e8 = np.zeros((256, 8), np.float32)
    for hd in range(256):
        e8[hd, hd // 32] = 1.0
    _add_lin(misc, "E8", e8.T)          # treat as w [8, 256]
    # Eexp: lhsT [8, 256] (broadcast head value to its 32 dims)
    _add_lin(misc, "Eexp", e8)          # w [256, 8] -> wT [8, 256] single slab
    misc.add("ones1", np.full((4, 128), 0.25, np.float32))

    laya = layb = None
    a_blobs, b_blobs = [], []
    for li in range(L):
        lp = p["layers"][li]
        mha = lp["mha"]
        pa = _Packer()
        _add_lin(pa, "f", mha["f"][0])
        _add_lin(pa, "e", mha["e"][0])
        _add_lin(pa, "v1", mha["v"][0])
        _add_lin(pa, "v2", mha["v"][2])
        _add_lin(pa, "w1", mha["w"][0])
        _add_lin(pa, "w2", mha["w"][2])
        _add_lin(pa, "m", mha["m"][0])
        _add_lin(pa, "a", mha["a"][0])
        pb2 = _Packer()
        _add_lin(pb2, "o", mha["o"][0])
        _add_lin(pb2, "mn1", lp["mn"][0])
        _add_lin(pb2, "mn2", lp["mn"][2])
        _add_lin(pb2, "me1", lp["me"][0])
        _add_lin(pb2, "me2", lp["me"][2])
        if laya is None:
            laya, layb = pa, pb2
        a_blobs.append(pa.blob())
        b_blobs.append(pb2.blob())
    layer_wa = np.stack(a_blobs)
    layer_wb = np.stack(b_blobs)

    bc = _Packer()
    # time-embedding constants: s/(2pi) and phase (0 or 0.25), packed per m-col
    scales = np.exp(np.arange(0, DC, 2, dtype=np.float64) * (-np.log(10000.0) / DC))
    sp = np.repeat(scales, 2) / (2.0 * np.pi)   # [256]
    ph = np.tile([0.0, 0.25], DC // 2)          # [256]
    _add_bias(bc, "eps", np.full(128, EPS))
    _add_bias(bc, "s2pi", sp)
    _add_bias(bc, "phase", ph)
    _add_bias(bc, "in_n_b1", p["in_n"][1])
    _add_bias(bc, "in_n_b2", p["in_n"][3])
    _add_bias(bc, "in_e_b1", p["in_e"][1])
    _add_bias(bc, "in_e_b2", p["in_e"][3])
    _add_bias(bc, "in_c_b1", p["in_c"][1])
    _add_bias(bc, "in_c_b2", p["in_c"][3])
    _add_bias(bc, "out_t_b1", p["out_t"][1])
    _add_bias(bc, "out_t_b2", p["out_t"][3])
    _add_bias(bc, "out_p_b1", p["out_p"][1])
    _add_bias(bc, "out_p_b2", p["out_p"][3])
    _add_bias(bc, "out_e_b1", p["out_e"][1])
    _add_bias(bc, "out_e_b2", p["out_e"][3])
    for li in range(L):
        lp = p["layers"][li]
        mha = lp["mha"]
        _add_bias(bc, f"l{li}_bf", mha["f"][1])
        _add_bias(bc, f"l{li}_be", mha["e"][1])
        _add_bias(bc, f"l{li}_bv1", mha["v"][1])
        _add_bias(bc, f"l{li}_bv2", mha["v"][3])
        _add_bias(bc, f"l{li}_bw1", mha["w"][1])
        _add_bias(bc, f"l{li}_bw2", mha["w"][3])
        _add_bias(bc, f"l{li}_bo", mha["o"][1])
        _add_bias(bc, f"l{li}_bm", mha["m"][1])
        _add_bias(bc, f"l{li}_ba", mha["a"][1])
        _add_bias(bc, f"l{li}_bmn1", lp["mn"][1])
        _add_bias(bc, f"l{li}_bmn2", lp["mn"][3])
        _add_bias(bc, f"l{li}_bme1", lp["me"][1])
        _add_bias(bc, f"l{li}_bme2", lp["me"][3])
        for nn_, wdt in (("n1", DN), ("n2", DE), ("n3", DN), ("n4", DE)):
            g, b_ = lp[nn_]
            _add_bias(bc, f"l{li}_{nn_}g", g)
            _add_bias(bc, f"l{li}_{nn_}b", b_)

    return misc, layer_wa, laya, layer_wb, layb, bc


def _build_program(misc_off, misc_cols, la_off, la_cols, lb_off, lb_cols, bc_off, bc_cols, taps=False):
    nc = bacc.Bacc("TRN2", debug=False)

    d_misc = nc.dram_tensor("misc_w", [128, misc_cols], F32R, kind="ExternalInput").ap()
    d_laya = nc.dram_tensor("layer_wa", [L, 128, la_cols], F32R, kind="ExternalInput").ap()
    d_layb = nc.dram_tensor("layer_wb", [L, 128, lb_cols], F32R, kind="ExternalInput").ap()
    d_bc = nc.dram_tensor("bias_c", [128, bc_cols], F32, kind="ExternalInput").ap()
    d_nodes = nc.dram_tensor("nodes_fm", [BLOC, NODE_D, N], F32R, kind="ExternalInput").ap()
    d_edges = nc.dram_tensor("edges_fm", [BLOC, 8, NN], F32R, kind="ExternalInput").ap()
    d_t = nc.dram_tensor("t_f32", [4, BLOC], F32R, kind="ExternalInput").ap()
    d_on = nc.dram_tensor("out_nodes_fm", [BLOC, NODE_D, N], F32, kind="ExternalOutput").ap()
    d_oe = nc.dram_tensor("out_edges_fm", [BLOC, EDGE_D, NN], F32, kind="ExternalOutput").ap()

    tapd = {}

    def tap(name, shape):
        if taps:
            tapd[name] = nc.dram_tensor(name, shape, F32, kind="ExternalOutput").ap()
        return tapd.get(name)

    with tile.TileContext(nc) as tc:
        with (
            tc.tile_pool(name="pw_misc", bufs=1) as pw_misc,
            tc.tile_pool(name="pw_laya", bufs=2) as pw_laya,
            tc.tile_pool(name="pw_layb", bufs=1) as pw_layb,
            tc.tile_pool(name="pb", bufs=1) as pbp,
            tc.tile_pool(name="pstate", bufs=1) as pstate,
            tc.tile_pool(name="pchunk", bufs=2) as pch,
            tc.tile_pool(name="pchunk1", bufs=1) as pch1,
            tc.tile_pool(name="psmall", bufs=2) as psm,
            tc.tile_pool(name="pseq", bufs=1) as psq,
            tc.tile_pool(name="pio", bufs=2) as pio,
            tc.tile_pool(name="pp", bufs=8, space="PSUM") as pp,
        ):
            mw = pw_misc.tile([128, misc_cols], F32R, tag="misc")
            nc.sync.dma_start(mw[:], d_misc[:])
            bcw = pbp.tile([128, bc_cols], F32, tag="bias")
            nc.sync.dma_start(bcw[:], d_bc[:])

            def MW(name, m=0, mw_width=128):
                off, w, h = misc_off[name]
                return mw[:h, off + m * mw_width : off + min((m + 1) * mw_width, w)]

            def BC(name, m=0, h=128):
                off, w, _ = bc_off[name]
                return bcw[:h, off + m : off + m + 1]

            def mm_psum(lhs_list, rhs_list, out_shape=(128, CF)):
                """Accumulating matmul chain into a fresh psum tile."""
                ps = pp.tile(list(out_shape), F32, tag="mm")
                nk = len(lhs_list)
                for k in range(nk):
                    nc.tensor.matmul(
                        ps[:], lhs_list[k], rhs_list[k],
                        start=(k == 0), stop=(k == nk - 1),
                    )
                return ps

            # ============ per-core program: loop over local batches ============
            for b in range(BLOC):
                # ---- persistent per-batch state tiles (reused across batches) --
                edges_t = pstate.tile([128, NN], F32R, tag="edges")
                ae_t = pstate.tile([128, NN], F32, tag="ae")
                nodes_t = psq.tile([128, 2, N], F32R, tag="nodes")

                # ---------------- time embedding + cond -----------------------
                t_sb = psm.tile([4, BLOC], F32R, tag="tsb")
                nc.sync.dma_start(t_sb[:], d_t[:])
                ps_t = pp.tile([128, BLOC], F32, tag="mm")
                nc.tensor.matmul(ps_t[:], MW("ones1"), t_sb[:], start=True, stop=True)
                emb = psq.tile([128, 2, BLOC], F32R, tag="emb")
                for m in range(2):
                    y = psm.tile([128, BLOC], F32, tag="y")
                    nc.scalar.activation(y[:], ps_t[:], A.Identity,
                                         bias=BC("phase", m), scale=BC("s2pi", m))
                    ki = psm.tile([128, BLOC], I32, tag="ki")
                    nc.vector.tensor_copy(ki[:], y[:])
                    kf = psm.tile([128, BLOC], F32, tag="kf")
                    nc.vector.tensor_copy(kf[:], ki[:])
                    fr = psm.tile([128, BLOC], F32, tag="fr")
                    nc.vector.tensor_sub(fr[:], y[:], kf[:])
                    nc.scalar.activation(emb[:, m, :], fr[:], A.Sin, scale=float(2 * np.pi))
                cmid = psq.tile([128, 2, BLOC], F32R, tag="cmid")
                for m in range(2):
                    ps = mm_psum([MW("in_c1_k0", m), MW("in_c1_k1", m)],
                                 [emb[:, 0, :], emb[:, 1, :]], (128, BLOC))
                    nc.scalar.activation(cmid[:, m, :], ps[:], A.Prelu,
                                         bias=BC("in_c_b1", m), alpha=ALPHA)
                cond = psq.tile([128, 2, BLOC], F32R, tag="cond")
                for m in range(2):
                    ps = mm_psum([MW("in_c2_k0", m), MW("in_c2_k1", m)],
                                 [cmid[:, 0, :], cmid[:, 1, :]], (128, BLOC))
                    nc.scalar.activation(cond[:, m, :], ps[:], A.Prelu,
                                         bias=BC("in_c_b2", m), alpha=ALPHA)
                if b == 0:
                    t_ = tap("tap_cond", [128, 2, BLOC])
                    if t_ is not None:
                        nc.sync.dma_start(t_[:], cond[:])

                # ---------------- input MLPs ---------------------------------
                nraw = psq.tile([NODE_D, N], F32R, tag="nraw")
                nc.sync.dma_start(nraw[:], d_nodes[b])
                nmid = psq.tile([128, 2, N], F32R, tag="nmid")
                for m in range(2):
                    ps = mm_psum([MW("in_n1_k0", m)], [nraw[:]], (128, N))
                    nc.scalar.activation(nmid[:, m, :], ps[:], A.Prelu,
                                         bias=BC("in_n_b1", m), alpha=ALPHA)
                for m in range(2):
                    ps = mm_psum([MW("in_n2_k0", m), MW("in_n2_k1", m)],
                                 [nmid[:, 0, :], nmid[:, 1, :]], (128, N))
                    nc.scalar.activation(nodes_t[:, m, :], ps[:], A.Prelu,
                                         bias=BC("in_n_b2", m), alpha=ALPHA)

                for s in range(NME):
                    sl = slice(s * ME_F, (s + 1) * ME_F)
                    eraw = pio.tile([8, ME_F], F32R, tag="pio_e")
                    nc.sync.dma_start(eraw[:], d_edges[b][:, sl])
                    ps = mm_psum([MW("in_e1_k0")], [eraw[:]], (128, ME_F))
                    emid = pio.tile([128, ME_F], F32R, tag="pio_r")
                    nc.scalar.activation(emid[:], ps[:], A.Prelu,
                                         bias=BC("in_e_b1"), alpha=ALPHA)
                    ps = mm_psum([MW("in_e2_k0")], [emid[:]], (128, ME_F))
                    nc.scalar.activation(edges_t[:, sl], ps[:], A.Prelu,
                                         bias=BC("in_e_b2"), alpha=ALPHA)

                if b == 0:
                    t_ = tap("tap_nodes_in", [128, 2, N])
                    if t_ is not None:
                        nc.sync.dma_start(t_[:], nodes_t[:])
                    t_ = tap("tap_edges_in", [128, 512])
                    if t_ is not None:
                        nc.sync.dma_start(t_[:], edges_t[:, 0:512])

                # ================= layers =====================================
                st_prev = None
                for li in range(L):
                    lwa = pw_laya.tile([128, la_cols], F32R, tag="laywa")
                    nc.sync.dma_start(lwa[:], d_laya[li])
                    lwb = pw_layb.tile([128, lb_cols], F32R, tag="laywb")
                    nc.sync.dma_start(lwb[:], d_layb[li])

                    def LW(name, m=0, _la=lwa, _lb=lwb):
                        if name.split("_k")[0] in ("o", "mn1", "mn2", "me1", "me2"):
                            off, w, h = lb_off[name]
                            return _lb[:h, off + m * 128 : off + min((m + 1) * 128, w)]
                        off, w, h = la_off[name]
                        return _la[:h, off + m * 128 : off + min((m + 1) * 128, w)]

                    pfx = f"l{li}_"

                    # ---- FiLM params: mul/add = cond @ Wm/Wa + b -------------
                    mul_sb = psm.tile([128, 2, BLOC], F32, tag="mul")
                    filmb = psm.tile([128, 2, BLOC], F32, tag="filmb")
                    add_sb = psm.tile([128, 2, BLOC], F32, tag="adds")
                    for m in range(2):
                        ps = mm_psum([LW("m_k0", m), LW("m_k1", m)],
                                     [cond[:, 0, :], cond[:, 1, :]], (128, BLOC))
                        nc.scalar.activation(mul_sb[:, m, :], ps[:], A.Identity,
                                             bias=BC(pfx + "bm", m))
                        ps = mm_psum([LW("a_k0", m), LW("a_k1", m)],
                                     [cond[:, 0, :], cond[:, 1, :]], (128, BLOC))
                        nc.scalar.activation(add_sb[:, m, :], ps[:], A.Identity,
                                             bias=BC(pfx + "ba", m))
                        # filmb = bf * mul + add  (per batch column)
                        for bb in range(BLOC):
                            nc.vector.tensor_scalar(
                                filmb[:, m, bb : bb + 1], BC(pfx + "bf", m),
                                mul_sb[:, m, bb : bb + 1], add_sb[:, m, bb : bb + 1],
                                op0=OP.mult, op1=OP.add)

                    # ---- chunk loop over i-rows ------------------------------
                    st1 = psq.tile([128, NCHUNK, 6], F32, tag="st1")
                    denom = psq.tile([8, N], F32, tag="denom")
                    out_raw = psq.tile([128, 2, N], F32, tag="oraw")
                    for c, (i0, ch) in enumerate(CHUNKS):
                        cf = ch * N
                        csl = slice(i0 * N, i0 * N + cf)
                        if li > 0:
                            # lazy n4 apply from previous layer: x2 -> edges
                            nc.scalar.activation(edges_t[:, csl], ae_t[:, csl],
                                                 A.Identity, bias=st_prev[:, 1:2],
                                                 scale=st_prev[:, 0:1])
                        # f = Prelu((gf @ Wf.T) * mul + (bf*mul+add))
                        f_sb = pch.tile([128, 2, cf], F32R, tag="f_sb")
                        for m in range(2):
                            lhs = [LW(f"f_k{k}", m) for k in range(5)]
                            rhs = [
                                edges_t[:, csl],
                                nodes_t[:, 0, :].unsqueeze(1).broadcast_to([128, ch, N]),
                                nodes_t[:, 1, :].unsqueeze(1).broadcast_to([128, ch, N]),
                                nodes_t[:, 0, i0 : i0 + ch].unsqueeze(2).broadcast_to([128, ch, N]),
                                nodes_t[:, 1, i0 : i0 + ch].unsqueeze(2).broadcast_to([128, ch, N]),
                            ]
                            ps = mm_psum(lhs, rhs, (128, cf))
                            nc.scalar.activation(f_sb[:, m, :], ps[:], A.Prelu,
                                                 bias=filmb[:, m, b : b + 1],
                                                 scale=mul_sb[:, m, b : b + 1], alpha=ALPHA)
                        if taps and b == 0 and li == 0 and c == 0:
                            t_ = tap("tap_f0", [128, 2, cf])
                            nc.vector.tensor_copy(tapdbg_f := pch.tile([128, 2, cf], F32, tag="tapf"), f_sb[:])
                            nc.sync.dma_start(t_[:], tapdbg_f[:])
                        # ae = Prelu(f @ We.T + be); x1 = edges + ae; stats
                        ps = mm_psum([LW("e_k0"), LW("e_k1")],
                                     [f_sb[:, 0, :], f_sb[:, 1, :]], (128, cf))
                        nc.scalar.activation(ae_t[:, csl], ps[:], A.Prelu,
                                             bias=BC(pfx + "be"), alpha=ALPHA)
                        nc.vector.tensor_add(ae_t[:, csl], edges_t[:, csl], ae_t[:, csl])
                        nc.vector.bn_stats(st1[:, c, :], ae_t[:, csl])
                        # v mlp
                        vmid = pch.tile([128, 2, cf], F32R, tag="vmid")
                        for m in range(2):
                            ps = mm_psum([LW("v1_k0", m), LW("v1_k1", m)],
                                         [f_sb[:, 0, :], f_sb[:, 1, :]], (128, cf))
                            nc.scalar.activation(vmid[:, m, :], ps[:], A.Prelu,
                                                 bias=BC(pfx + "bv1", m), alpha=ALPHA)
                        v_sb = pch.tile([128, 2, cf], F32, tag="v_sb")
                        for m in range(2):
                            ps = mm_psum([LW("v2_k0", m), LW("v2_k1", m)],
                                         [vmid[:, 0, :], vmid[:, 1, :]], (128, cf))
                            nc.vector.tensor_scalar(v_sb[:, m, :], ps[:],
                                                    BC(pfx + "bv2", m), 0.0,
                                                    op0=OP.add, op1=OP.add)
                        # w mlp -> w^2
                        wmid = pch.tile([128, 2, cf], F32R, tag="wmid")
                        for m in range(2):
                            ps = mm_psum([LW("w1_k0", m), LW("w1_k1", m)],
                                         [f_sb[:, 0, :], f_sb[:, 1, :]], (128, cf))
                            nc.scalar.activation(wmid[:, m, :], ps[:], A.Prelu,
                                                 bias=BC(pfx + "bw1", m), alpha=ALPHA)
                        w2sb = pch.tile([128, 2, cf], F32R, tag="w2sb")
                        for m in range(2):
                            ps = mm_psum([LW("w2_k0", m), LW("w2_k1", m)],
                                         [wmid[:, 0, :], wmid[:, 1, :]], (128, cf))
                            nc.scalar.activation(w2sb[:, m, :], ps[:], A.Square,
                                                 bias=BC(pfx + "bw2", m))
                        # wn2 -> exp(sqrt(q/256))
                        ps_wn = mm_psum([MW("E8_k0"), MW("E8_k1")],
                                        [w2sb[:, 0, :], w2sb[:, 1, :]], (8, cf))
                        expc = pch.tile([8, cf], F32R, tag="expc")
                        nc.scalar.activation(expc[:], ps_wn[:], A.Ln, scale=1.0 / DN)
                        nc.scalar.activation(expc[:], expc[:], A.Exp, scale=0.5)
                        nc.scalar.activation(expc[:], expc[:], A.Exp)
                        nc.vector.tensor_reduce(
                            denom[:, i0 : i0 + ch],
                            expc[:].rearrange("p (i j) -> p i j", i=ch),
                            axis=AX.X, op=OP.add)
                        # attention accumulate: out_raw += sum_j exp * v
                        for m in range(2):
                            ps_x = pp.tile([128, cf], F32, tag="mm")
                            nc.tensor.matmul(ps_x[:], MW("Eexp_k0", m), expc[:],
                                             start=True, stop=True)
                            pv = pch1.tile([128, cf], F32, tag="pv")
                            nc.vector.tensor_mul(pv[:], ps_x[:], v_sb[:, m, :])
                            nc.vector.tensor_reduce(
                                out_raw[:, m, i0 : i0 + ch],
                                pv[:].rearrange("p (i j) -> p i j", i=ch),
                                axis=AX.X, op=OP.add)

                    # ---- attention finalize + node path ----------------------
                    lnden = psm.tile([8, N], F32, tag="lnden")
                    nc.scalar.activation(lnden[:], denom[:], A.Ln)
                    rden = psm.tile([8, N], F32R, tag="rden")
                    nc.scalar.activation(rden[:], lnden[:], A.Exp, scale=-1.0)
                    att = psq.tile([128, 2, N], F32R, tag="att")
                    for m in range(2):
                        ps_r = pp.tile([128, N], F32, tag="mm")
                        nc.tensor.matmul(ps_r[:], MW("Eexp_k0", m), rden[:],
                                         start=True, stop=True)
                        nc.vector.tensor_mul(att[:, m, :], out_raw[:, m, :], ps_r[:])
                    an = psq.tile([128, 2, N], F32, tag="an")
                    for m in range(2):
                        ps = mm_psum([LW("o_k0", m), LW("o_k1", m)],
                                     [att[:, 0, :], att[:, 1, :]], (128, N))
                        nc.scalar.activation(an[:, m, :], ps[:], A.Prelu,
                                             bias=BC(pfx + "bo", m), alpha=ALPHA)
                    if taps and b == 0 and li == 0:
                        t_ = tap("tap_att", [128, 2, N])
                        nc.sync.dma_start(t_[:], an[:])
                        t_ = tap("tap_denom", [8, N])
                        nc.sync.dma_start(t_[:], denom[:])

                    def node_norm(x_t, gname, bname):
                        """Instance-norm x_t [128,2,N] over N per feature -> nodes_t."""
                        for m in range(2):
                            stn = psm.tile([128, 6], F32, tag="stn")
                            nc.vector.bn_stats(stn[:], x_t[:, m, :])
                            mv = psm.tile([128, 2], F32, tag="mvn")
                            nc.vector.bn_aggr(mv[:], stn[:])
                            lnv = psm.tile([128, 1], F32, tag="lnvn")
                            nc.scalar.activation(lnv[:], mv[:, 1:2], A.Ln, bias=BC("eps"))
                            rstd = psm.tile([128, 1], F32, tag="rstdn")
                            nc.scalar.activation(rstd[:], lnv[:], A.Exp, scale=-0.5)
                            s_ = psm.tile([128, 1], F32, tag="sn")
                            nc.vector.tensor_mul(s_[:], rstd[:], BC(gname, m))
                            ms = psm.tile([128, 1], F32, tag="msn")
                            nc.vector.tensor_mul(ms[:], mv[:, 0:1], s_[:])
                            tt_ = psm.tile([128, 1], F32, tag="tn")
                            nc.vector.tensor_sub(tt_[:], BC(bname, m), ms[:])
                            nc.scalar.activation(nodes_t[:, m, :], x_t[:, m, :],
                                                 A.Identity, bias=tt_[:, 0:1],
                                                 scale=s_[:, 0:1])

                    x1n = psq.tile([128, 2, N], F32, tag="x1n")
                    nc.vector.tensor_add(x1n[:], nodes_t[:], an[:])
                    node_norm(x1n, pfx + "n1g", pfx + "n1b")

                    nmid2 = psq.tile([128, 2, N], F32R, tag="nmid2")
                    for m in range(2):
                        ps = mm_psum([LW("mn1_k0", m), LW("mn1_k1", m)],
                                     [nodes_t[:, 0, :], nodes_t[:, 1, :]], (128, N))
                        nc.scalar.activation(nmid2[:, m, :], ps[:], A.Prelu,
                                             bias=BC(pfx + "bmn1", m), alpha=ALPHA)
                    x2n = psq.tile([128, 2, N], F32, tag="x2n")
                    for m in range(2):
                        ps = mm_psum([LW("mn2_k0", m), LW("mn2_k1", m)],
                                     [nmid2[:, 0, :], nmid2[:, 1, :]], (128, N))
                        tmp = psm.tile([128, N], F32, tag="tmpn")
                        nc.scalar.activation(tmp[:], ps[:], A.Prelu,
                                             bias=BC(pfx + "bmn2", m), alpha=ALPHA)
                        nc.vector.tensor_add(x2n[:, m, :], nodes_t[:, m, :], tmp[:])
                    node_norm(x2n, pfx + "n3g", pfx + "n3b")

                    # ---- edge norm scale/bias from stats (apply is lazy) -----
                    def edge_norm_st(stats_tile, gname, bname, role):
                        mv = psm.tile([128, 2], F32, tag="mve")
                        nc.vector.bn_aggr(mv[:], stats_tile[:])
                        lnv = psm.tile([128, 1], F32, tag="lnve")
                        nc.scalar.activation(lnv[:], mv[:, 1:2], A.Ln, bias=BC("eps"))
                        rstd = psm.tile([128, 1], F32, tag="rstde")
                        nc.scalar.activation(rstd[:], lnv[:], A.Exp, scale=-0.5)
                        st_ = psm.tile([128, 2], F32, tag="st" + role)
                        nc.vector.tensor_mul(st_[:, 0:1], rstd[:], BC(gname))
                        ms = psm.tile([128, 1], F32, tag="mse")
                        nc.vector.tensor_mul(ms[:], mv[:, 0:1], st_[:, 0:1])
                        nc.vector.tensor_sub(st_[:, 1:2], BC(bname), ms[:])
                        return st_

                    st_n2 = edge_norm_st(st1, pfx + "n2g", pfx + "n2b", "n2")

                    # ---- me mlp + residual + n4 (n2 applied lazily per slice) -
                    st2 = psq.tile([128, NME, 6], F32, tag="st2")
                    for s in range(NME):
                        sl = slice(s * ME_F, (s + 1) * ME_F)
                        nc.scalar.activation(edges_t[:, sl], ae_t[:, sl], A.Identity,
                                             bias=st_n2[:, 1:2], scale=st_n2[:, 0:1])
                        ps = mm_psum([LW("me1_k0")], [edges_t[:, sl]], (128, ME_F))
                        memid = pio.tile([128, ME_F], F32R, tag="pio_r")
                        nc.scalar.activation(memid[:], ps[:], A.Prelu,
                                             bias=BC(pfx + "bme1"), alpha=ALPHA)
                        ps = mm_psum([LW("me2_k0")], [memid[:]], (128, ME_F))
                        meout = pio.tile([128, ME_F], F32, tag="pio_f")
                        nc.scalar.activation(meout[:], ps[:], A.Prelu,
                                             bias=BC(pfx + "bme2"), alpha=ALPHA)
                        nc.vector.tensor_add(ae_t[:, sl], edges_t[:, sl], meout[:])
                        nc.vector.bn_stats(st2[:, s, :], ae_t[:, sl])
                    st_prev = edge_norm_st(st2, pfx + "n4g", pfx + "n4b", "n4")

                    if taps and b == 0 and li == 0:
                        t_ = tap("tap_nodes_l0", [128, 2, N])
                        nc.sync.dma_start(t_[:], nodes_t[:])
                        t_ = tap("tap_edges_l0", [128, 512])
                        nc.sync.dma_start(t_[:], edges_t[:, 0:512])

                # ================= output heads ===============================
                for w1n, w2n, b1n, b2n, odim, opad, obase in (
                    ("out_t1", "out_t2", "out_t_b1", "out_t_b2", 6, 8, 0),
                    ("out_p1", "out_p2", "out_p_b1", "out_p_b2", 14, 16, 6),
                ):
                    hmid = psq.tile([128, 2, N], F32R, tag="hmid")
                    for m in range(2):
                        ps = mm_psum([MW(f"{w1n}_k0", m), MW(f"{w1n}_k1", m)],
                                     [nodes_t[:, 0, :], nodes_t[:, 1, :]], (128, N))
                        nc.scalar.activation(hmid[:, m, :], ps[:], A.Prelu,
                                             bias=BC(b1n, m), alpha=ALPHA)
                    ps = mm_psum([MW(f"{w2n}_k0"), MW(f"{w2n}_k1")],
                                 [hmid[:, 0, :], hmid[:, 1, :]], (opad, N))
                    no_sb = psq.tile([14, N], F32, tag="nosb" + w1n)
                    nc.scalar.activation(no_sb[:odim, :], ps[:odim, :],
                                         A.Identity, bias=BC(b2n, h=odim))
                    nc.sync.dma_start(d_on[b][obase : obase + odim, :], no_sb[:odim, :])

                for s in range(NME):
                    sl = slice(s * ME_F, (s + 1) * ME_F)
                    nc.scalar.activation(edges_t[:, sl], ae_t[:, sl], A.Identity,
                                         bias=st_prev[:, 1:2], scale=st_prev[:, 0:1])
                    ps = mm_psum([MW("out_e1_k0")], [edges_t[:, sl]], (128, ME_F))
                    oemid = pio.tile([128, ME_F], F32R, tag="pio_r")
                    nc.scalar.activation(oemid[:], ps[:], A.Prelu,
                                         bias=BC("out_e_b1"), alpha=ALPHA)
                    ps = mm_psum([MW("out_e2_k0")], [oemid[:]], (8, ME_F))
                    oe_sb = pio.tile([EDGE_D, ME_F], F32, tag="pio_o")
                    nc.scalar.activation(oe_sb[:], ps[:EDGE_D, :], A.Identity,
                                         bias=BC("out_e_b2", h=EDGE_D))
                    nc.sync.dma_start(d_oe[b][:, sl], oe_sb[:])

    import bass_rust as _br
    from concourse.hw_specs import get_activation_tables

    def _act_tables_pinned():
        tables = list(get_activation_tables(nc.m.arch).items())
        keep = {"natural_log_exp_and_others", "trig_and_small"}
        _br.insert_act_table_loads(
            nc, [(n, (f if n in keep else set())) for n, f in tables])

    nc.insert_act_table_loads = _act_tables_pinned
    nc.compile()
    return nc


_PROG_CACHE = {}


def _get_program(misc_off, misc_cols, la_off, la_cols, lb_off, lb_cols, bc_off, bc_cols, taps=False):
    key = (misc_cols, la_cols, lb_cols, bc_cols, taps)
    if key not in _PROG_CACHE:
        _PROG_CACHE[key] = _build_program(
            misc_off, misc_cols, la_off, la_cols, lb_off, lb_cols, bc_off, bc_cols, taps=taps)
    return _PROG_CACHE[key]


def kernel(nodes, edges, timestep, params, _taps=False, _trace=False):
    global LAST_RESULTS
    nodes = np.asarray(nodes, np.float32)
    edges = np.asarray(edges, np.float32)
    timestep = np.asarray(timestep)

    misc, layer_wa, laya, layer_wb, layb, bc = _pack_host(params)
    misc_blob = misc.blob()
    bc_blob = bc.blob()

    nc = _get_program(misc.off, misc_blob.shape[1], laya.off, layer_wa.shape[2],
                      layb.off, layer_wb.shape[2], bc.off, bc_blob.shape[1], taps=_taps)

    nodes_fm = np.ascontiguousarray(nodes.transpose(0, 2, 1))          # [B,20,96]
    edges_fm = np.zeros((B, 8, NN), np.float32)
    edges_fm[:, :EDGE_D, :] = edges.reshape(B, NN, EDGE_D).transpose(0, 2, 1)
    t_f32 = timestep.astype(np.float32)

    in_maps = []
    for c in range(NCORES):
        bs = slice(c * BLOC, (c + 1) * BLOC)
        in_maps.append({
            "misc_w": misc_blob,
            "layer_wa": layer_wa,
            "layer_wb": layer_wb,
            "bias_c": bc_blob,
            "nodes_fm": nodes_fm[bs],
            "edges_fm": edges_fm[bs],
            "t_f32": np.broadcast_to(t_f32[bs], (4, BLOC)).copy(),
        })

    res = bass_utils.run_bass_kernel_spmd(
        nc, in_maps, core_ids=list(range(NCORES)), trace=_trace)
    LAST_RESULTS = res

    on = np.concatenate([r["out_nodes_fm"] for r in res.results])      # [B,20,96]
    oe = np.concatenate([r["out_edges_fm"] for r in res.results])      # [B,5,9216]
    out_nodes = np.ascontiguousarray(on.transpose(0, 2, 1))            # [B,96,20]
    out_edges = np.ascontiguousarray(
        oe.transpose(0, 2, 1).reshape(B, N, N, EDGE_D))
    return out_nodes, out_edges
